# revision 51
# baseline (speedup 1.0000x reference)
"""EvolveGCN (2-layer) Trainium2 Bass kernel, 8-way sharded.

Algebraic reduction: the mat-GRU evolving the GCN weights is data-independent
and only h2[T-1] is returned, so the whole model collapses to

    W1* = matGRU^4(W1);  W2* = matGRU^4(W2)      (tiny host math)
    h1  = rrelu(A3 @ (X3 @ W1*));  out = rrelu(A3 @ (h1 @ W2*))

Device schedule (per core, nodes range-partitioned by original id):
  - X arrives transposed bf16 [128F, RTP]; table build is a plain matmul
    lhsT=xsT slice (even/odd row split so the fp16 DRAM shard writes are
    512B-contiguous), PSUM->fp16 via Activation copy.
  - AllGather replicates the fp16 table [50176, 128] to every core.
  - SWDGE dma_gather pulls per-edge messages (one 256B descriptor per edge)
    group A (table rows < 5*RTP) / group B split so indices fit int16.
  - Segment-sum runs on the tensor engine: per 64-row window, PSUM
    accumulates accT[128F, 64rows] += msg_chunk.T @ S_chunk, where S
    [128 edge-slots, 64 rows] carries val at (slot, row).  S is built
    on-device by the vector engine from packed val/rr arrays
    (S = (iota == rr) * val with 0-stride broadcast APs), not DMAed.
  - rrelu + down-cast is a single Prelu activation; layer-1 windows land in
    a transposed bf16 h1T tile that directly feeds the layer-2 table build
    (interleaved with layer-1's spmm); layer-2 windows land in a transposed
    fp16 out tile, written back per segment.
  - Host packs rows into windows (LPT on per-row A/B in-degree) so nearly
    every (window, group) hits its chunk budget exactly; the shared SPMD
    schedule is the per-window max over cores.
"""

import sys
import numpy as np

for _p in ("/opt/trn_rl_repo",):
    if _p not in sys.path:
        sys.path.insert(0, _p)

from ml_dtypes import bfloat16 as np_bf16

T, N, E, F = 4, 50000, 800000, 128
NC = 8
NPC = N // NC            # 6250 nodes per core
RTP = 6272               # padded rows per core (49 tiles of 128)
NT = RTP // 128          # 49 row tiles per core
WROWS = 64               # scatter window rows
NW = RTP // WROWS        # 98 windows per core
ACORES = 5               # table rows of cores [0,5) are group A
SPLIT = ACORES * RTP     # 31360 < 32768: both groups' indices fit int16
SLOPE = 11.0 / 48.0      # torch RReLU eval negative slope
SEGP = 2                 # row tiles per gather segment
TGT_A = 640              # per-window group-A edge target (5 chunks)
TGT_B = 384              # per-window group-B edge target (3 chunks)

SIM1 = False  # single-core, no-collective variant for TimelineSim
REPS = 1


def _evolve(W0, gW, gU, gb, steps=T):
    def sig(x):
        return 1.0 / (1.0 + np.exp(-x))

    Q = W0.astype(np.float64)
    gW = gW.astype(np.float64)
    gU = gU.astype(np.float64)
    gb = gb.astype(np.float64)
    for _ in range(steps):
        z = sig(gW[0] @ Q + gU[0] @ Q + gb[0])
        r = sig(gW[1] @ Q + gU[1] @ Q + gb[1])
        h = np.tanh(gW[2] @ Q + gU[2] @ (r * Q) + gb[2])
        Q = (1.0 - z) * Q + z * h
    return Q.astype(np.float32)


def _pack_windows(a, b, capA, capB, rng, wa=3, wb=5):
    """Assign rows (with group in-degrees a, b) of one shard to NW windows of
    64 slots, keeping window sums <= (capA[w], capB[w]).  Snake-deal by
    degree, then pairwise swap-repair of violations.  Returns positions."""
    n = len(a)
    order = np.argsort(-(a * wa + b * wb), kind="stable")
    wins = np.empty(n, np.int64)
    rnds = np.arange(n) // NW
    js = np.arange(n) % NW
    wins[order] = np.where(rnds % 2 == 0, js, NW - 1 - js)

    def sums():
        A = np.bincount(wins, weights=a, minlength=NW).astype(np.int64)
        B = np.bincount(wins, weights=b, minlength=NW).astype(np.int64)
        return A, B

    A, B = sums()
    members = [list(np.nonzero(wins == w)[0]) for w in range(NW)]
    al = a.tolist()
    bl = b.tolist()
    capAl, capBl = capA.tolist(), capB.tolist()
    stuck = np.zeros(NW, bool)
    resets = 0
    for _it in range(20000):
        vA = np.maximum(A - capA, 0)
        vB = np.maximum(B - capB, 0)
        v = vA + vB
        va = v.copy()
        va[stuck] = 0
        if va.max() == 0:
            if v.max() == 0 or stuck.all() or resets >= 6:
                break
            stuck[:] = False
            resets += 1
            continue
        w = int(np.argmax(va))
        overA = bool(vA[w] > 0)
        overB = bool(vB[w] > 0)
        rw = members[w]
        sc_w = sorted(rw, key=lambda r: -(al[r] * overA + bl[r] * overB))[:10]
        roomA = capA - A
        roomB = capB - B
        cand_w2 = np.argpartition(-(roomA + roomB), 10)[:10]
        cand_w2 = cand_w2[np.argsort(-(roomA + roomB)[cand_w2])]
        done = False
        for r in sc_w:
            ar, br = al[r], bl[r]
            for w2 in cand_w2:
                if w2 == w:
                    continue
                w2 = int(w2)
                r2i = sorted(
                    members[w2],
                    key=lambda x: al[x] * overA + bl[x] * overB,
                )[:10]
                vold = int(v[w] + v[w2])
                for r2 in r2i:
                    a2, b2 = al[r2], bl[r2]
                    nA_w, nB_w = A[w] - ar + a2, B[w] - br + b2
                    nA_2, nB_2 = A[w2] + ar - a2, B[w2] + br - b2
                    new = (max(nA_w - capAl[w], 0) + max(nB_w - capBl[w], 0)
                           + max(nA_2 - capAl[w2], 0) + max(nB_2 - capBl[w2], 0))
                    if new < vold:
                        wins[r], wins[r2] = w2, w
                        members[w].remove(r)
                        members[w2].remove(r2)
                        members[w].append(r2)
                        members[w2].append(r)
                        A[w], B[w] = nA_w, nB_w
                        A[w2], B[w2] = nA_2, nB_2
                        done = True
                        break
                if done:
                    break
            if done:
                break
        if not done:
            stuck[w] = True
    pos = np.empty(n, np.int64)
    for w in range(NW):
        rows = np.nonzero(wins == w)[0]
        pos[rows] = w * WROWS + np.arange(len(rows))
    return pos


def _prep_edges(row, col, val):
    """Host-side schedule. Returns (sched, per-core input arrays)."""
    # ---- window packing -> within-shard positions
    gcol = (col // NPC) >= ACORES
    a_deg = np.bincount(row[~gcol], minlength=N)
    b_deg = np.bincount(row[gcol], minlength=N)
    # shared overflow-window profile: last KA/KB windows get one extra chunk
    a_tot = a_deg.reshape(NC, NPC).sum(axis=1)
    b_tot = b_deg.reshape(NC, NPC).sum(axis=1)
    KA = max(0, -(-(int(a_tot.max()) + 256 - NW * TGT_A) // 128))
    KB = max(0, -(-(int(b_tot.max()) + 256 - NW * TGT_B) // 128))
    capA = np.full(NW, TGT_A, np.int64)
    capA[NW - KA :] = TGT_A + 128
    capB = np.full(NW, TGT_B, np.int64)
    capB[NW - KB :] = TGT_B + 128
    pos = np.empty(N, np.int64)
    rng = np.random.default_rng(0)
    for i in range(NC):
        lo, hi = i * NPC, (i + 1) * NPC
        best = None
        for wa, wb in ((3, 5), (1, 1), (5, 3), (1, 3)):
            p = _pack_windows(
                a_deg[lo:hi], b_deg[lo:hi], capA, capB, rng, wa, wb
            )
            w = p // WROWS
            A = np.bincount(w, weights=a_deg[lo:hi], minlength=NW)
            B = np.bincount(w, weights=b_deg[lo:hi], minlength=NW)
            score = (
                np.maximum(-(-A.astype(np.int64) // 128) - capA // 128, 0).sum()
                + np.maximum(-(-B.astype(np.int64) // 128) - capB // 128, 0).sum()
            )
            if best is None or score < best[0]:
                best = (score, p)
            if score == 0:
                break
        pos[lo:hi] = best[1]

    corei = row // NPC
    rl = pos[row]                       # scatter position within shard
    win = rl // WROWS
    rr = rl % WROWS
    tcol = (col // NPC) * RTP + pos[col]  # table row
    grp = (tcol >= SPLIT).astype(np.int64)

    # ---- merge exact duplicate (row, col) edges (S can only route a slot
    # to one destination row, so merging is valid only for identical rows)
    key = row * np.int64(N) + col
    order = np.argsort(key, kind="stable")
    key_s = key[order]
    uniq = np.empty(len(key_s), bool)
    uniq[0] = True
    uniq[1:] = key_s[1:] != key_s[:-1]
    seg_id = np.cumsum(uniq) - 1
    val_m = np.bincount(seg_id, weights=val[order].astype(np.float64))
    first = order[uniq]
    corei, win, rr, tcol, grp = (
        corei[first], win[first], rr[first], tcol[first], grp[first])
    val_m = val_m.astype(np.float32)

    # ---- shared chunk schedule: per (grp, win) max over cores
    counts = np.zeros((NC, 2, NW), np.int64)
    np.add.at(counts, (corei, grp, win), 1)
    CC = -(-counts // 128)
    CC = CC.max(axis=0)                 # [2, NW]
    CC[0] = np.maximum(CC[0], 1)        # every window needs >= 1 chunk
    baseA = np.zeros(NW + 1, np.int64)
    baseA[1:] = np.cumsum(CC[0])
    baseB = np.zeros(NW + 1, np.int64)
    baseB[1:] = np.cumsum(CC[1])
    NCHA, NCHB = int(baseA[-1]), int(baseB[-1])
    NCH = NCHA + NCHB
    NA, NB = NCHA * 128, NCHB * 128
    # unified S chunk ids, window-major (A then B within each window) so the
    # DVE S-build completes chunks in the order the spmm consumes them
    offW = np.zeros(NW + 1, np.int64)
    offW[1:] = np.cumsum(CC[0] + CC[1])

    idxa = np.zeros((NC, 128, NA // 16), np.int16)
    idxb = np.zeros((NC, 128, NB // 16), np.int16)
    valp = np.zeros((NC, 128, NCH), np.float16)
    rrp = np.full((NC, 128, NCH), 127.0, np.float16)

    for i in range(NC):
        for g, (base, idxg, idxoff) in enumerate(
            ((baseA, idxa, 0), (baseB, idxb, SPLIT))
        ):
            m = (corei == i) & (grp == g)
            ew, err = win[m], rr[m]
            etc = (tcol[m] - idxoff).astype(np.int16)
            ev = val_m[m]
            o = np.argsort(ew, kind="stable")
            ew, err, etc, ev = ew[o], err[o], etc[o], ev[o]
            winstart = np.searchsorted(ew, np.arange(NW))
            slot = base[ew] * 128 + (np.arange(ew.size) - winstart[ew])
            assert (slot < base[ew + 1] * 128).all()
            flat = np.zeros(base[-1] * 128, np.int16)
            flat[slot] = etc
            idxg[i][:16] = flat.reshape(-1, 16).T
            idxg[i] = np.tile(idxg[i][:16], (8, 1))
            p = slot % 128
            # unified chunk id: window-major
            gch = slot // 128                    # group-major chunk id
            loc = gch - base[ew]                 # chunk within window
            ch = offW[ew] + g * CC[0][ew] + loc
            valp[i, p, ch] = ev.astype(np.float16)
            rrp[i, p, ch] = err.astype(np.float16)

    sched = dict(
        CC=CC, baseA=baseA, baseB=baseB, NCHA=NCHA, NCHB=NCHB, offW=offW
    )
    return sched, pos, idxa, idxb, valp, rrp


def _build_program(sched):
    import concourse.bass as bass
    import concourse.tile as tile
    from concourse import bacc, mybir
    from contextlib import ExitStack

    F32, F16, BF16, I16 = (
        mybir.dt.float32, mybir.dt.float16, mybir.dt.bfloat16, mybir.dt.int16)
    baseA, baseB = sched["baseA"], sched["baseB"]
    NCHA, NCHB = sched["NCHA"], sched["NCHB"]
    offW = sched["offW"]
    CCA = sched["CC"][0]
    NCH = NCHA + NCHB
    NA, NB = NCHA * 128, NCHB * 128

    nc = bacc.Bacc(
        "TRN2", target_bir_lowering=False, debug=False,
        num_devices=(1 if SIM1 else NC),
    )
    xst_d = nc.dram_tensor("xst", [F, RTP], BF16, kind="ExternalInput")
    w1_d = nc.dram_tensor("w1", [F, F], BF16, kind="ExternalInput")
    w2_d = nc.dram_tensor("w2", [F, F], BF16, kind="ExternalInput")
    iota_d = nc.dram_tensor("iota", [128, WROWS], F16, kind="ExternalInput")
    idxa_d = nc.dram_tensor("idxa", [128, NA // 16], I16, kind="ExternalInput")
    idxb_d = nc.dram_tensor("idxb", [128, NB // 16], I16, kind="ExternalInput")
    valp_d = nc.dram_tensor("valp", [128, NCH], F16, kind="ExternalInput")
    rrp_d = nc.dram_tensor("rrp", [128, NCH], F16, kind="ExternalInput")
    out_d = nc.dram_tensor("out", [F, RTP], F16, kind="ExternalOutput")

    # gather segments: SEGP row tiles each
    WQ = 128 // WROWS
    segs = []
    for p0 in range(0, NT, SEGP):
        p1 = min(p0 + SEGP, NT)
        segs.append((p0, p1, p0 * WQ, p1 * WQ))
    max_cha = max(int(baseA[w1] - baseA[w0]) for _, _, w0, w1 in segs)
    max_chb = max(int(baseB[w1] - baseB[w0]) for _, _, w0, w1 in segs)

    with tile.TileContext(nc) as tc, ExitStack() as ctx:
        const = ctx.enter_context(tc.tile_pool(name="const", bufs=1))
        big = ctx.enter_context(tc.tile_pool(name="big", bufs=1))
        tps = ctx.enter_context(tc.tile_pool(name="tps", bufs=2, space="PSUM"))
        tsh = ctx.enter_context(tc.tile_pool(name="tsh", bufs=14))
        accp = ctx.enter_context(tc.tile_pool(name="accp", bufs=4, space="PSUM"))
        msgp = ctx.enter_context(tc.tile_pool(name="msgp", bufs=3))
        h1p = ctx.enter_context(tc.tile_pool(name="h1p", bufs=3))
        dram = ctx.enter_context(tc.tile_pool(name="dram", bufs=1, space="DRAM"))

        # table-build / allgather chunks (tile ranges), segment-aligned,
        # small tail chunk so the layer transition drains fast
        CHB = [0, 10, 20, 28, 36, 44, NT]
        NCHK = len(CHB) - 1

        # --- inputs with no deps first: fill DMA idle during table build
        w1_sb = const.tile([F, F], BF16)
        nc.sync.dma_start(w1_sb[:], w1_d[:, :])
        w2_sb = const.tile([F, F], BF16)
        nc.sync.dma_start(w2_sb[:], w2_d[:, :])
        xst_c = []
        for g in range(NCHK):
            t0, t1 = CHB[g], CHB[g + 1]
            xt = big.tile([F, (t1 - t0) * 128], BF16, name=f"xst{g}")
            nc.sync.dma_start(xt[:], xst_d[:, t0 * 128 : t1 * 128])
            xst_c.append(xt)
        iota_sb = const.tile([128, WROWS], F16)
        nc.sync.dma_start(iota_sb[:], iota_d[:, :])
        idxa_sb = big.tile([128, NA // 16], I16)
        nc.sync.dma_start(idxa_sb[:], idxa_d[:, :])
        idxb_sb = big.tile([128, NB // 16], I16)
        nc.sync.dma_start(idxb_sb[:], idxb_d[:, :])
        valp_sb = big.tile([128, NCH], F16)
        nc.sync.dma_start(valp_sb[:], valp_d[:, :])
        rrp_sb = big.tile([128, NCH], F16)
        nc.sync.dma_start(rrp_sb[:], rrp_d[:, :])

        s_sb = big.tile([128, NCH * WROWS], F16)

        def build_s():
            # S[p, cid*64 + j] = (iota[j] == rr[p,cid]) * val[p,cid], on DVE
            SLAB = 128
            for c0 in range(0, NCH, SLAB):
                c1 = min(c0 + SLAB, NCH)
                nch = c1 - c0
                s_slab = s_sb[:, c0 * WROWS : c1 * WROWS]
                s3 = s_slab.rearrange("p (c j) -> p c j", j=WROWS)
                iota_b = iota_sb[:, :].unsqueeze(1).broadcast_to([128, nch, WROWS])
                rr_b = rrp_sb[:, c0:c1].unsqueeze(2).broadcast_to([128, nch, WROWS])
                val_b = valp_sb[:, c0:c1].unsqueeze(2).broadcast_to([128, nch, WROWS])
                nc.vector.tensor_tensor(
                    out=s3, in0=iota_b, in1=rr_b, op=mybir.AluOpType.is_equal
                )
                nc.vector.tensor_tensor(
                    out=s3, in0=s3, in1=val_b, op=mybir.AluOpType.mult
                )

        def build_tiles(src_sb, src_t0, w_sb, shard, t0, t1, dma_eng=None):
            """table rows [t0*128, t1*128) = (src^T)[rows] @ w; one [128,128]
            matmul per tile, up to four tiles per copy/DMA.  dma_eng: queue
            for the shard writes (layer 1 uses Pool so the writes aren't
            stuck behind the input loads on the in-order SP queue)."""
            sh3 = shard.rearrange("(t q b) -> q t b", q=64, b=256)
            t = t0
            while t < t1:
                grp = min(4, t1 - t)
                ps = tps.tile([64, 1024], F32, tag="tp")
                for k in range(grp):
                    s0 = (t + k - src_t0) * 128
                    for par in range(2):
                        nc.tensor.matmul(
                            out=ps[:, k * 256 + par * 128 : k * 256 + (par + 1) * 128],
                            lhsT=src_sb[:, s0 + par : s0 + 128 : 2],
                            rhs=w_sb[:],
                            start=True, stop=True,
                        )
                sh = tsh.tile([64, 1024], F16, tag="sh")
                nc.scalar.activation(
                    sh[:, : grp * 256], ps[:, : grp * 256],
                    mybir.ActivationFunctionType.Copy,
                )
                sh_t = sh.rearrange("p (t b) -> p t b", b=256)
                (dma_eng or nc.sync).dma_start(
                    sh3[:, t : t + grp, :],
                    sh_t[:, :grp, :],
                )
                t += grp

        def all_gather(shard, table):
            if SIM1:
                for r in range(NC):
                    nc.sync.dma_start(
                        table[r * RTP * F : (r + 1) * RTP * F], shard[:]
                    )
            else:
                nc.gpsimd.collective_compute(
                    "AllGather",
                    mybir.AluOpType.bypass,
                    replica_groups=[list(range(NC))],
                    ins=[shard.opt()],
                    outs=[table.opt()],
                )

        # --- layer-1 table build (chunked for pipelining) + allgather
        _aspace = "Local" if SIM1 else "Shared"
        shard1 = dram.tile([RTP * F], F16, name="shard1")
        shard2 = dram.tile([RTP * F], F16, name="shard2")
        table1 = dram.tile([NC * RTP * F], F16, addr_space=_aspace, name="table1")
        table2 = dram.tile([NC * RTP * F], F16, addr_space=_aspace, name="table2")
        for g in range(NCHK):
            build_tiles(
                xst_c[g], CHB[g], w1_sb, shard1, CHB[g], CHB[g + 1],
            )
        build_s()
        all_gather(shard1, table1)

        def spmm(table, emit, interleave=None, per_tile=None):
            tbl = table.rearrange("(r f) -> r f", f=F)
            for si, (p0, p1, w0, w1) in enumerate(segs):
                ca0, ca1 = int(baseA[w0]), int(baseA[w1])
                cb0, cb1 = int(baseB[w0]), int(baseB[w1])
                na, nb = (ca1 - ca0) * 128, (cb1 - cb0) * 128
                msga = msgp.tile([128, max_cha, 128], F16, tag="msga")
                msgb = msgp.tile([128, max_chb, 128], F16, tag="msgb")
                if na:
                    nc.gpsimd.dma_gather(
                        out_ap=msga[:, : ca1 - ca0, :],
                        in_ap=tbl[:SPLIT, :],
                        idxs_ap=idxa_sb[:, ca0 * 8 : ca1 * 8],
                        num_idxs=na,
                        num_idxs_reg=na,
                        elem_size=F,
                        single_packet=False,
                    )
                if nb:
                    nc.gpsimd.dma_gather(
                        out_ap=msgb[:, : cb1 - cb0, :],
                        in_ap=tbl[SPLIT:, :],
                        idxs_ap=idxb_sb[:, cb0 * 8 : cb1 * 8],
                        num_idxs=nb,
                        num_idxs_reg=nb,
                        elem_size=F,
                        single_packet=False,
                    )
                emt = emit(si)
                for w in range(w0, w1):
                    acc = accp.tile([128, WROWS], F32, tag="acc")
                    nw_ch = int(
                        baseA[w + 1] - baseA[w] + baseB[w + 1] - baseB[w]
                    )
                    k = 0
                    for gc in range(int(baseA[w]), int(baseA[w + 1])):
                        cid = int(offW[w]) + (gc - int(baseA[w]))
                        nc.tensor.matmul(
                            out=acc[:],
                            lhsT=msga[:, gc - ca0, :],
                            rhs=s_sb[:, cid * WROWS : (cid + 1) * WROWS],
                            start=(k == 0),
                            stop=(k == nw_ch - 1),
                        )
                        k += 1
                    for gc in range(int(baseB[w]), int(baseB[w + 1])):
                        cid = int(offW[w]) + int(CCA[w]) + (gc - int(baseB[w]))
                        nc.tensor.matmul(
                            out=acc[:],
                            lhsT=msgb[:, gc - cb0, :],
                            rhs=s_sb[:, cid * WROWS : (cid + 1) * WROWS],
                            start=(k == 0),
                            stop=(k == nw_ch - 1),
                        )
                        k += 1
                    emt(w - w0, acc)
                    if per_tile is not None and w % WQ == WQ - 1:
                        per_tile(si, p0, w // WQ)
                if interleave is not None:
                    interleave(si, p0, p1)

        # --- layer 1: spmm -> h1T (bf16, per-segment tiles) -> table2 build
        h1tiles = {}

        def emit1(si):
            h1t = h1p.tile([F, SEGP * 128], BF16, tag="h1t")
            h1tiles[si] = h1t

            def e(wloc, acc):
                nc.scalar.activation(
                    h1t[:, wloc * WROWS : (wloc + 1) * WROWS],
                    acc[:],
                    mybir.ActivationFunctionType.Prelu,
                    alpha=SLOPE,
                )
            return e

        def interleave1(si, p0, p1):
            build_tiles(h1tiles[si], p0, w2_sb, shard2, p0, p1)

        spmm(table1, emit1, interleave1)
        all_gather(shard2, table2)

        # --- layer 2: spmm -> outT fp16 -> DRAM per segment
        out_sb = big.tile([F, RTP], F16)

        def emit2(si):
            p0, p1, w0, w1 = segs[si]

            def e(wloc, acc):
                w = w0 + wloc
                nc.scalar.activation(
                    out_sb[:, w * WROWS : (w + 1) * WROWS],
                    acc[:],
                    mybir.ActivationFunctionType.Prelu,
                    alpha=SLOPE,
                )
            return e

        def interleave2(si, p0, p1):
            nc.sync.dma_start(
                out_d[:, p0 * 128 : p1 * 128],
                out_sb[:, p0 * 128 : p1 * 128],
            )

        spmm(table2, emit2, interleave2)

    nc.compile()
    return nc


def kernel(
    features,
    adj_row,
    adj_col,
    adj_val,
    W1,
    g1_W,
    g1_U,
    g1_b,
    W2,
    g2_W,
    g2_U,
    g2_b,
    _run_kwargs=None,
):
    from concourse.bass_utils import run_bass_kernel_spmd

    X = np.asarray(features[T - 1], dtype=np.float32)
    row = np.asarray(adj_row[T - 1], dtype=np.int64)
    col = np.asarray(adj_col[T - 1], dtype=np.int64)
    val = np.asarray(adj_val[T - 1], dtype=np.float32)

    W1f = _evolve(np.asarray(W1), np.asarray(g1_W), np.asarray(g1_U), np.asarray(g1_b))
    W2f = _evolve(np.asarray(W2), np.asarray(g2_W), np.asarray(g2_U), np.asarray(g2_b))

    sched, pos, idxa, idxb, valp, rrp = _prep_edges(row, col, val)
    nc = _build_program(sched)

    # xsT per core: [128, RTP] bf16, column pos[v] = X[v]
    xst = np.zeros((NC, F, RTP), np_bf16)
    for i in range(NC):
        lo, hi = i * NPC, (i + 1) * NPC
        xst[i][:, pos[lo:hi]] = X[lo:hi].T.astype(np_bf16)

    iota = np.broadcast_to(
        np.arange(WROWS, dtype=np.float16), (128, WROWS)
    ).copy()

    in_maps = [
        {
            "xst": xst[i],
            "w1": W1f.astype(np_bf16),
            "w2": W2f.astype(np_bf16),
            "iota": iota,
            "idxa": idxa[i],
            "idxb": idxb[i],
            "valp": valp[i],
            "rrp": rrp[i],
        }
        for i in range(NC)
    ]
    res = run_bass_kernel_spmd(
        nc, in_maps, core_ids=list(range(NC)), **(_run_kwargs or {})
    )
    out = np.empty((N, F), np.float32)
    for i in range(NC):
        lo, hi = i * NPC, (i + 1) * NPC
        arr = res.results[i]["out"].astype(np.float32)  # [F, RTP]
        out[lo:hi] = arr[:, pos[lo:hi]].T
    if _run_kwargs:
        kernel.last_results = res
    return out


# revision 53
# speedup vs baseline: 1.0014x; 1.0014x over previous
"""EvolveGCN (2-layer) Trainium2 Bass kernel, 8-way sharded.

Algebraic reduction: the mat-GRU evolving the GCN weights is data-independent
and only h2[T-1] is returned, so the whole model collapses to

    W1* = matGRU^4(W1);  W2* = matGRU^4(W2)      (tiny host math)
    h1  = rrelu(A3 @ (X3 @ W1*));  out = rrelu(A3 @ (h1 @ W2*))

Device schedule (per core, nodes range-partitioned by original id):
  - X arrives transposed bf16 [128F, RTP]; table build is a plain matmul
    lhsT=xsT slice (even/odd row split so the fp16 DRAM shard writes are
    512B-contiguous), PSUM->fp16 via Activation copy.
  - AllGather replicates the fp16 table [50176, 128] to every core.
  - SWDGE dma_gather pulls per-edge messages (one 256B descriptor per edge)
    group A (table rows < 5*RTP) / group B split so indices fit int16.
  - Segment-sum runs on the tensor engine: per 64-row window, PSUM
    accumulates accT[128F, 64rows] += msg_chunk.T @ S_chunk, where S
    [128 edge-slots, 64 rows] carries val at (slot, row).  S is built
    on-device by the vector engine from packed val/rr arrays
    (S = (iota == rr) * val with 0-stride broadcast APs), not DMAed.
  - rrelu + down-cast is a single Prelu activation; layer-1 windows land in
    a transposed bf16 h1T tile that directly feeds the layer-2 table build
    (interleaved with layer-1's spmm); layer-2 windows land in a transposed
    fp16 out tile, written back per segment.
  - Host packs rows into windows (LPT on per-row A/B in-degree) so nearly
    every (window, group) hits its chunk budget exactly; the shared SPMD
    schedule is the per-window max over cores.
"""

import sys
import numpy as np

for _p in ("/opt/trn_rl_repo",):
    if _p not in sys.path:
        sys.path.insert(0, _p)

from ml_dtypes import bfloat16 as np_bf16

T, N, E, F = 4, 50000, 800000, 128
NC = 8
NPC = N // NC            # 6250 nodes per core
RTP = 6272               # padded rows per core (49 tiles of 128)
NT = RTP // 128          # 49 row tiles per core
WROWS = 64               # scatter window rows
NW = RTP // WROWS        # 98 windows per core
ACORES = 5               # table rows of cores [0,5) are group A
SPLIT = ACORES * RTP     # 31360 < 32768: both groups' indices fit int16
SLOPE = 11.0 / 48.0      # torch RReLU eval negative slope
SEGP = 2                 # row tiles per gather segment
TGT_A = 640              # per-window group-A edge target (5 chunks)
TGT_B = 384              # per-window group-B edge target (3 chunks)

SIM1 = False  # single-core, no-collective variant for TimelineSim
REPS = 1


def _evolve(W0, gW, gU, gb, steps=T):
    def sig(x):
        return 1.0 / (1.0 + np.exp(-x))

    Q = W0.astype(np.float64)
    gW = gW.astype(np.float64)
    gU = gU.astype(np.float64)
    gb = gb.astype(np.float64)
    for _ in range(steps):
        z = sig(gW[0] @ Q + gU[0] @ Q + gb[0])
        r = sig(gW[1] @ Q + gU[1] @ Q + gb[1])
        h = np.tanh(gW[2] @ Q + gU[2] @ (r * Q) + gb[2])
        Q = (1.0 - z) * Q + z * h
    return Q.astype(np.float32)


def _pack_windows(a, b, capA, capB, rng, wa=3, wb=5):
    """Assign rows (with group in-degrees a, b) of one shard to NW windows of
    64 slots, keeping window sums <= (capA[w], capB[w]).  Snake-deal by
    degree, then pairwise swap-repair of violations.  Returns positions."""
    n = len(a)
    order = np.argsort(-(a * wa + b * wb), kind="stable")
    wins = np.empty(n, np.int64)
    rnds = np.arange(n) // NW
    js = np.arange(n) % NW
    wins[order] = np.where(rnds % 2 == 0, js, NW - 1 - js)

    def sums():
        A = np.bincount(wins, weights=a, minlength=NW).astype(np.int64)
        B = np.bincount(wins, weights=b, minlength=NW).astype(np.int64)
        return A, B

    A, B = sums()
    members = [list(np.nonzero(wins == w)[0]) for w in range(NW)]
    al = a.tolist()
    bl = b.tolist()
    capAl, capBl = capA.tolist(), capB.tolist()
    stuck = np.zeros(NW, bool)
    resets = 0
    for _it in range(20000):
        vA = np.maximum(A - capA, 0)
        vB = np.maximum(B - capB, 0)
        v = vA + vB
        va = v.copy()
        va[stuck] = 0
        if va.max() == 0:
            if v.max() == 0 or stuck.all() or resets >= 6:
                break
            stuck[:] = False
            resets += 1
            continue
        w = int(np.argmax(va))
        overA = bool(vA[w] > 0)
        overB = bool(vB[w] > 0)
        rw = members[w]
        sc_w = sorted(rw, key=lambda r: -(al[r] * overA + bl[r] * overB))[:10]
        roomA = capA - A
        roomB = capB - B
        cand_w2 = np.argpartition(-(roomA + roomB), 10)[:10]
        cand_w2 = cand_w2[np.argsort(-(roomA + roomB)[cand_w2])]
        done = False
        for r in sc_w:
            ar, br = al[r], bl[r]
            for w2 in cand_w2:
                if w2 == w:
                    continue
                w2 = int(w2)
                r2i = sorted(
                    members[w2],
                    key=lambda x: al[x] * overA + bl[x] * overB,
                )[:10]
                vold = int(v[w] + v[w2])
                for r2 in r2i:
                    a2, b2 = al[r2], bl[r2]
                    nA_w, nB_w = A[w] - ar + a2, B[w] - br + b2
                    nA_2, nB_2 = A[w2] + ar - a2, B[w2] + br - b2
                    new = (max(nA_w - capAl[w], 0) + max(nB_w - capBl[w], 0)
                           + max(nA_2 - capAl[w2], 0) + max(nB_2 - capBl[w2], 0))
                    if new < vold:
                        wins[r], wins[r2] = w2, w
                        members[w].remove(r)
                        members[w2].remove(r2)
                        members[w].append(r2)
                        members[w2].append(r)
                        A[w], B[w] = nA_w, nB_w
                        A[w2], B[w2] = nA_2, nB_2
                        done = True
                        break
                if done:
                    break
            if done:
                break
        if not done:
            stuck[w] = True
    pos = np.empty(n, np.int64)
    for w in range(NW):
        rows = np.nonzero(wins == w)[0]
        pos[rows] = w * WROWS + np.arange(len(rows))
    return pos


def _prep_edges(row, col, val):
    """Host-side schedule. Returns (sched, per-core input arrays)."""
    # ---- window packing -> within-shard positions
    gcol = (col // NPC) >= ACORES
    a_deg = np.bincount(row[~gcol], minlength=N)
    b_deg = np.bincount(row[gcol], minlength=N)
    # shared overflow-window profile: last KA/KB windows get one extra chunk
    a_tot = a_deg.reshape(NC, NPC).sum(axis=1)
    b_tot = b_deg.reshape(NC, NPC).sum(axis=1)
    KA = max(0, -(-(int(a_tot.max()) + 256 - NW * TGT_A) // 128))
    KB = max(0, -(-(int(b_tot.max()) + 256 - NW * TGT_B) // 128))
    capA = np.full(NW, TGT_A, np.int64)
    capA[NW - KA :] = TGT_A + 128
    capB = np.full(NW, TGT_B, np.int64)
    capB[NW - KB :] = TGT_B + 128
    pos = np.empty(N, np.int64)
    rng = np.random.default_rng(0)
    for i in range(NC):
        lo, hi = i * NPC, (i + 1) * NPC
        best = None
        for wa, wb in ((3, 5), (1, 1), (5, 3), (1, 3)):
            p = _pack_windows(
                a_deg[lo:hi], b_deg[lo:hi], capA, capB, rng, wa, wb
            )
            w = p // WROWS
            A = np.bincount(w, weights=a_deg[lo:hi], minlength=NW)
            B = np.bincount(w, weights=b_deg[lo:hi], minlength=NW)
            score = (
                np.maximum(-(-A.astype(np.int64) // 128) - capA // 128, 0).sum()
                + np.maximum(-(-B.astype(np.int64) // 128) - capB // 128, 0).sum()
            )
            if best is None or score < best[0]:
                best = (score, p)
            if score == 0:
                break
        pos[lo:hi] = best[1]

    corei = row // NPC
    rl = pos[row]                       # scatter position within shard
    win = rl // WROWS
    rr = rl % WROWS
    tcol = (col // NPC) * RTP + pos[col]  # table row
    grp = (tcol >= SPLIT).astype(np.int64)

    # ---- merge exact duplicate (row, col) edges (S can only route a slot
    # to one destination row, so merging is valid only for identical rows)
    key = row * np.int64(N) + col
    order = np.argsort(key, kind="stable")
    key_s = key[order]
    uniq = np.empty(len(key_s), bool)
    uniq[0] = True
    uniq[1:] = key_s[1:] != key_s[:-1]
    seg_id = np.cumsum(uniq) - 1
    val_m = np.bincount(seg_id, weights=val[order].astype(np.float64))
    first = order[uniq]
    corei, win, rr, tcol, grp = (
        corei[first], win[first], rr[first], tcol[first], grp[first])
    val_m = val_m.astype(np.float32)

    # ---- shared chunk schedule: per (grp, win) max over cores
    counts = np.zeros((NC, 2, NW), np.int64)
    np.add.at(counts, (corei, grp, win), 1)
    CC = -(-counts // 128)
    CC = CC.max(axis=0)                 # [2, NW]
    CC[0] = np.maximum(CC[0], 1)        # every window needs >= 1 chunk
    baseA = np.zeros(NW + 1, np.int64)
    baseA[1:] = np.cumsum(CC[0])
    baseB = np.zeros(NW + 1, np.int64)
    baseB[1:] = np.cumsum(CC[1])
    NCHA, NCHB = int(baseA[-1]), int(baseB[-1])
    NCH = NCHA + NCHB
    NA, NB = NCHA * 128, NCHB * 128
    # unified S chunk ids, window-major (A then B within each window) so the
    # DVE S-build completes chunks in the order the spmm consumes them
    offW = np.zeros(NW + 1, np.int64)
    offW[1:] = np.cumsum(CC[0] + CC[1])

    idxa = np.zeros((NC, 128, NA // 16), np.int16)
    idxb = np.zeros((NC, 128, NB // 16), np.int16)
    valp = np.zeros((NC, 128, NCH), np.float16)
    rrp = np.full((NC, 128, NCH), 127.0, np.float16)

    for i in range(NC):
        for g, (base, idxg, idxoff) in enumerate(
            ((baseA, idxa, 0), (baseB, idxb, SPLIT))
        ):
            m = (corei == i) & (grp == g)
            ew, err = win[m], rr[m]
            etc = (tcol[m] - idxoff).astype(np.int16)
            ev = val_m[m]
            o = np.argsort(ew, kind="stable")
            ew, err, etc, ev = ew[o], err[o], etc[o], ev[o]
            winstart = np.searchsorted(ew, np.arange(NW))
            slot = base[ew] * 128 + (np.arange(ew.size) - winstart[ew])
            assert (slot < base[ew + 1] * 128).all()
            flat = np.zeros(base[-1] * 128, np.int16)
            flat[slot] = etc
            idxg[i][:16] = flat.reshape(-1, 16).T
            idxg[i] = np.tile(idxg[i][:16], (8, 1))
            p = slot % 128
            # unified chunk id: window-major
            gch = slot // 128                    # group-major chunk id
            loc = gch - base[ew]                 # chunk within window
            ch = offW[ew] + g * CC[0][ew] + loc
            valp[i, p, ch] = ev.astype(np.float16)
            rrp[i, p, ch] = err.astype(np.float16)

    sched = dict(
        CC=CC, baseA=baseA, baseB=baseB, NCHA=NCHA, NCHB=NCHB, offW=offW
    )
    return sched, pos, idxa, idxb, valp, rrp


def _build_program(sched):
    import concourse.bass as bass
    import concourse.tile as tile
    from concourse import bacc, mybir
    from contextlib import ExitStack

    F32, F16, BF16, I16 = (
        mybir.dt.float32, mybir.dt.float16, mybir.dt.bfloat16, mybir.dt.int16)
    baseA, baseB = sched["baseA"], sched["baseB"]
    NCHA, NCHB = sched["NCHA"], sched["NCHB"]
    offW = sched["offW"]
    CCA = sched["CC"][0]
    NCH = NCHA + NCHB
    NA, NB = NCHA * 128, NCHB * 128

    nc = bacc.Bacc(
        "TRN2", target_bir_lowering=False, debug=False,
        num_devices=(1 if SIM1 else NC),
    )
    xst_d = nc.dram_tensor("xst", [F, RTP], BF16, kind="ExternalInput")
    wio_d = nc.dram_tensor("wio", [F, 2 * F + WROWS], F16, kind="ExternalInput")
    idxab_d = nc.dram_tensor(
        "idxab", [128, (NA + NB) // 16], I16, kind="ExternalInput"
    )
    vr_d = nc.dram_tensor("vr", [128, 2 * NCH], F16, kind="ExternalInput")
    out_d = nc.dram_tensor("out", [F, RTP], F16, kind="ExternalOutput")

    # gather segments: SEGP row tiles each
    WQ = 128 // WROWS
    segs = []
    for p0 in range(0, NT, SEGP):
        p1 = min(p0 + SEGP, NT)
        segs.append((p0, p1, p0 * WQ, p1 * WQ))
    max_cha = max(int(baseA[w1] - baseA[w0]) for _, _, w0, w1 in segs)
    max_chb = max(int(baseB[w1] - baseB[w0]) for _, _, w0, w1 in segs)

    with tile.TileContext(nc) as tc, ExitStack() as ctx:
        const = ctx.enter_context(tc.tile_pool(name="const", bufs=1))
        big = ctx.enter_context(tc.tile_pool(name="big", bufs=1))
        tps = ctx.enter_context(tc.tile_pool(name="tps", bufs=2, space="PSUM"))
        tsh = ctx.enter_context(tc.tile_pool(name="tsh", bufs=14))
        accp = ctx.enter_context(tc.tile_pool(name="accp", bufs=4, space="PSUM"))
        msgp = ctx.enter_context(tc.tile_pool(name="msgp", bufs=3))
        h1p = ctx.enter_context(tc.tile_pool(name="h1p", bufs=3))
        dram = ctx.enter_context(tc.tile_pool(name="dram", bufs=1, space="DRAM"))

        # table-build / allgather chunks (tile ranges), segment-aligned,
        # small tail chunk so the layer transition drains fast
        CHB = [0, 10, 20, 28, 36, 44, NT]
        NCHK = len(CHB) - 1

        # --- inputs with no deps first (merged to few DMAs: each issue
        # holds the HWDGE unit ~625ns): fill DMA idle during table build
        wio_sb = const.tile([F, 2 * F + WROWS], F16)
        nc.sync.dma_start(wio_sb[:], wio_d[:, :])
        w1_sb = wio_sb[:, 0:F].bitcast(BF16)
        w2_sb = wio_sb[:, F : 2 * F].bitcast(BF16)
        iota_sb = wio_sb[:, 2 * F : 2 * F + WROWS]
        xst_c = []
        for g in range(NCHK):
            t0, t1 = CHB[g], CHB[g + 1]
            xt = big.tile([F, (t1 - t0) * 128], BF16, name=f"xst{g}")
            nc.sync.dma_start(xt[:], xst_d[:, t0 * 128 : t1 * 128])
            xst_c.append(xt)
        idxab_sb = big.tile([128, (NA + NB) // 16], I16)
        nc.sync.dma_start(idxab_sb[:], idxab_d[:, :])
        idxa_sb = idxab_sb[:, : NA // 16]
        idxb_sb = idxab_sb[:, NA // 16 :]
        vr_sb = big.tile([128, 2 * NCH], F16)
        nc.sync.dma_start(vr_sb[:], vr_d[:, :])
        valp_sb = vr_sb[:, :NCH]
        rrp_sb = vr_sb[:, NCH:]

        s_sb = big.tile([128, NCH * WROWS], F16)

        def build_s():
            # S[p, cid*64 + j] = (iota[j] == rr[p,cid]) * val[p,cid], on DVE
            SLAB = 128
            for c0 in range(0, NCH, SLAB):
                c1 = min(c0 + SLAB, NCH)
                nch = c1 - c0
                s_slab = s_sb[:, c0 * WROWS : c1 * WROWS]
                s3 = s_slab.rearrange("p (c j) -> p c j", j=WROWS)
                iota_b = iota_sb.unsqueeze(1).broadcast_to([128, nch, WROWS])
                rr_b = rrp_sb[:, c0:c1].unsqueeze(2).broadcast_to([128, nch, WROWS])
                val_b = valp_sb[:, c0:c1].unsqueeze(2).broadcast_to([128, nch, WROWS])
                nc.vector.tensor_tensor(
                    out=s3, in0=iota_b, in1=rr_b, op=mybir.AluOpType.is_equal
                )
                nc.vector.tensor_tensor(
                    out=s3, in0=s3, in1=val_b, op=mybir.AluOpType.mult
                )

        def build_tiles(src_sb, src_t0, w_sb, shard, t0, t1, dma_eng=None):
            """table rows [t0*128, t1*128) = (src^T)[rows] @ w; one [128,128]
            matmul per tile, up to four tiles per copy/DMA.  dma_eng: queue
            for the shard writes (layer 1 uses Pool so the writes aren't
            stuck behind the input loads on the in-order SP queue)."""
            sh3 = shard.rearrange("(t q b) -> q t b", q=64, b=256)
            t = t0
            while t < t1:
                grp = min(4, t1 - t)
                ps = tps.tile([64, 1024], F32, tag="tp")
                for k in range(grp):
                    s0 = (t + k - src_t0) * 128
                    for par in range(2):
                        nc.tensor.matmul(
                            out=ps[:, k * 256 + par * 128 : k * 256 + (par + 1) * 128],
                            lhsT=src_sb[:, s0 + par : s0 + 128 : 2],
                            rhs=w_sb,
                            start=True, stop=True,
                        )
                sh = tsh.tile([64, 1024], F16, tag="sh")
                nc.scalar.activation(
                    sh[:, : grp * 256], ps[:, : grp * 256],
                    mybir.ActivationFunctionType.Copy,
                )
                sh_t = sh.rearrange("p (t b) -> p t b", b=256)
                (dma_eng or nc.sync).dma_start(
                    sh3[:, t : t + grp, :],
                    sh_t[:, :grp, :],
                )
                t += grp

        def all_gather(shard, table):
            if SIM1:
                for r in range(NC):
                    nc.sync.dma_start(
                        table[r * RTP * F : (r + 1) * RTP * F], shard[:]
                    )
            else:
                nc.gpsimd.collective_compute(
                    "AllGather",
                    mybir.AluOpType.bypass,
                    replica_groups=[list(range(NC))],
                    ins=[shard.opt()],
                    outs=[table.opt()],
                )

        # --- layer-1 table build (chunked for pipelining) + allgather
        _aspace = "Local" if SIM1 else "Shared"
        shard1 = dram.tile([RTP * F], F16, name="shard1")
        shard2 = dram.tile([RTP * F], F16, name="shard2")
        table1 = dram.tile([NC * RTP * F], F16, addr_space=_aspace, name="table1")
        table2 = dram.tile([NC * RTP * F], F16, addr_space=_aspace, name="table2")
        for g in range(NCHK):
            build_tiles(
                xst_c[g], CHB[g], w1_sb, shard1, CHB[g], CHB[g + 1],
            )
        build_s()
        all_gather(shard1, table1)

        def spmm(table, emit, interleave=None, per_tile=None):
            tbl = table.rearrange("(r f) -> r f", f=F)
            for si, (p0, p1, w0, w1) in enumerate(segs):
                ca0, ca1 = int(baseA[w0]), int(baseA[w1])
                cb0, cb1 = int(baseB[w0]), int(baseB[w1])
                na, nb = (ca1 - ca0) * 128, (cb1 - cb0) * 128
                msga = msgp.tile([128, max_cha, 128], F16, tag="msga")
                msgb = msgp.tile([128, max_chb, 128], F16, tag="msgb")
                if na:
                    nc.gpsimd.dma_gather(
                        out_ap=msga[:, : ca1 - ca0, :],
                        in_ap=tbl[:SPLIT, :],
                        idxs_ap=idxa_sb[:, ca0 * 8 : ca1 * 8],
                        num_idxs=na,
                        num_idxs_reg=na,
                        elem_size=F,
                        single_packet=False,
                    )
                if nb:
                    nc.gpsimd.dma_gather(
                        out_ap=msgb[:, : cb1 - cb0, :],
                        in_ap=tbl[SPLIT:, :],
                        idxs_ap=idxb_sb[:, cb0 * 8 : cb1 * 8],
                        num_idxs=nb,
                        num_idxs_reg=nb,
                        elem_size=F,
                        single_packet=False,
                    )
                emt = emit(si)
                for w in range(w0, w1):
                    acc = accp.tile([128, WROWS], F32, tag="acc")
                    nw_ch = int(
                        baseA[w + 1] - baseA[w] + baseB[w + 1] - baseB[w]
                    )
                    k = 0
                    for gc in range(int(baseA[w]), int(baseA[w + 1])):
                        cid = int(offW[w]) + (gc - int(baseA[w]))
                        nc.tensor.matmul(
                            out=acc[:],
                            lhsT=msga[:, gc - ca0, :],
                            rhs=s_sb[:, cid * WROWS : (cid + 1) * WROWS],
                            start=(k == 0),
                            stop=(k == nw_ch - 1),
                        )
                        k += 1
                    for gc in range(int(baseB[w]), int(baseB[w + 1])):
                        cid = int(offW[w]) + int(CCA[w]) + (gc - int(baseB[w]))
                        nc.tensor.matmul(
                            out=acc[:],
                            lhsT=msgb[:, gc - cb0, :],
                            rhs=s_sb[:, cid * WROWS : (cid + 1) * WROWS],
                            start=(k == 0),
                            stop=(k == nw_ch - 1),
                        )
                        k += 1
                    emt(w - w0, acc)
                    if per_tile is not None and w % WQ == WQ - 1:
                        per_tile(si, p0, w // WQ)
                if interleave is not None:
                    interleave(si, p0, p1)

        # --- layer 1: spmm -> h1T (bf16, per-segment tiles) -> table2 build
        h1tiles = {}

        def emit1(si):
            h1t = h1p.tile([F, SEGP * 128], BF16, tag="h1t")
            h1tiles[si] = h1t

            def e(wloc, acc):
                nc.scalar.activation(
                    h1t[:, wloc * WROWS : (wloc + 1) * WROWS],
                    acc[:],
                    mybir.ActivationFunctionType.Prelu,
                    alpha=SLOPE,
                )
            return e

        def interleave1(si, p0, p1):
            build_tiles(h1tiles[si], p0, w2_sb, shard2, p0, p1)

        spmm(table1, emit1, interleave1)
        all_gather(shard2, table2)

        # --- layer 2: spmm -> outT fp16 -> DRAM per segment
        out_sb = big.tile([F, RTP], F16)

        def emit2(si):
            p0, p1, w0, w1 = segs[si]

            def e(wloc, acc):
                w = w0 + wloc
                nc.scalar.activation(
                    out_sb[:, w * WROWS : (w + 1) * WROWS],
                    acc[:],
                    mybir.ActivationFunctionType.Prelu,
                    alpha=SLOPE,
                )
            return e

        def interleave2(si, p0, p1):
            nc.sync.dma_start(
                out_d[:, p0 * 128 : p1 * 128],
                out_sb[:, p0 * 128 : p1 * 128],
            )

        spmm(table2, emit2, interleave2)

    nc.compile()
    return nc


def kernel(
    features,
    adj_row,
    adj_col,
    adj_val,
    W1,
    g1_W,
    g1_U,
    g1_b,
    W2,
    g2_W,
    g2_U,
    g2_b,
    _run_kwargs=None,
):
    from concourse.bass_utils import run_bass_kernel_spmd

    X = np.asarray(features[T - 1], dtype=np.float32)
    row = np.asarray(adj_row[T - 1], dtype=np.int64)
    col = np.asarray(adj_col[T - 1], dtype=np.int64)
    val = np.asarray(adj_val[T - 1], dtype=np.float32)

    W1f = _evolve(np.asarray(W1), np.asarray(g1_W), np.asarray(g1_U), np.asarray(g1_b))
    W2f = _evolve(np.asarray(W2), np.asarray(g2_W), np.asarray(g2_U), np.asarray(g2_b))

    sched, pos, idxa, idxb, valp, rrp = _prep_edges(row, col, val)
    nc = _build_program(sched)

    # xsT per core: [128, RTP] bf16, column pos[v] = X[v]
    xst = np.zeros((NC, F, RTP), np_bf16)
    for i in range(NC):
        lo, hi = i * NPC, (i + 1) * NPC
        xst[i][:, pos[lo:hi]] = X[lo:hi].T.astype(np_bf16)

    # merged small inputs: [w1|w2] as bf16 bits in an f16 carrier + iota
    wio = np.zeros((F, 2 * F + WROWS), np.float16)
    wio[:, :F] = W1f.astype(np_bf16).view(np.float16)
    wio[:, F : 2 * F] = W2f.astype(np_bf16).view(np.float16)
    wio[:, 2 * F :] = np.arange(WROWS, dtype=np.float16)[None, :]
    idxab = np.concatenate([idxa, idxb], axis=2)
    vr = np.concatenate([valp, rrp], axis=2)

    in_maps = [
        {
            "xst": xst[i],
            "wio": wio,
            "idxab": idxab[i],
            "vr": vr[i],
        }
        for i in range(NC)
    ]
    res = run_bass_kernel_spmd(
        nc, in_maps, core_ids=list(range(NC)), **(_run_kwargs or {})
    )
    out = np.empty((N, F), np.float32)
    for i in range(NC):
        lo, hi = i * NPC, (i + 1) * NPC
        arr = res.results[i]["out"].astype(np.float32)  # [F, RTP]
        out[lo:hi] = arr[:, pos[lo:hi]].T
    if _run_kwargs:
        kernel.last_results = res
    return out


# revision 57
# speedup vs baseline: 1.0025x; 1.0011x over previous
"""EvolveGCN (2-layer) Trainium2 Bass kernel, 8-way sharded.

Algebraic reduction: the mat-GRU evolving the GCN weights is data-independent
and only h2[T-1] is returned, so the whole model collapses to

    W1* = matGRU^4(W1);  W2* = matGRU^4(W2)      (tiny host math)
    h1  = rrelu(A3 @ (X3 @ W1*));  out = rrelu(A3 @ (h1 @ W2*))

Device schedule (per core, nodes range-partitioned by original id):
  - X arrives transposed bf16 [128F, RTP]; table build is a plain matmul
    lhsT=xsT slice (even/odd row split so the fp16 DRAM shard writes are
    512B-contiguous), PSUM->fp16 via Activation copy.
  - AllGather replicates the fp16 table [50176, 128] to every core.
  - SWDGE dma_gather pulls per-edge messages (one 256B descriptor per edge)
    group A (table rows < 5*RTP) / group B split so indices fit int16.
  - Segment-sum runs on the tensor engine: per 64-row window, PSUM
    accumulates accT[128F, 64rows] += msg_chunk.T @ S_chunk, where S
    [128 edge-slots, 64 rows] carries val at (slot, row).  S is built
    on-device by the vector engine from packed val/rr arrays
    (S = (iota == rr) * val with 0-stride broadcast APs), not DMAed.
  - rrelu + down-cast is a single Prelu activation; layer-1 windows land in
    a transposed bf16 h1T tile that directly feeds the layer-2 table build
    (interleaved with layer-1's spmm); layer-2 windows land in a transposed
    fp16 out tile, written back per segment.
  - Host packs rows into windows (LPT on per-row A/B in-degree) so nearly
    every (window, group) hits its chunk budget exactly; the shared SPMD
    schedule is the per-window max over cores.
"""

import sys
import numpy as np

for _p in ("/opt/trn_rl_repo",):
    if _p not in sys.path:
        sys.path.insert(0, _p)

from ml_dtypes import bfloat16 as np_bf16

T, N, E, F = 4, 50000, 800000, 128
NC = 8
NPC = N // NC            # 6250 nodes per core
RTP = 6272               # padded rows per core (49 tiles of 128)
NT = RTP // 128          # 49 row tiles per core
WROWS = 64               # scatter window rows
NW = RTP // WROWS        # 98 windows per core
ACORES = 5               # table rows of cores [0,5) are group A
SPLIT = ACORES * RTP     # 31360 < 32768: both groups' indices fit int16
SLOPE = 11.0 / 48.0      # torch RReLU eval negative slope
SEGP = 2                 # row tiles per gather segment
TGT_A = 640              # per-window group-A edge target (5 chunks)
TGT_B = 384              # per-window group-B edge target (3 chunks)

SIM1 = False  # single-core, no-collective variant for TimelineSim
REPS = 1


def _evolve(W0, gW, gU, gb, steps=T):
    def sig(x):
        return 1.0 / (1.0 + np.exp(-x))

    Q = W0.astype(np.float64)
    gW = gW.astype(np.float64)
    gU = gU.astype(np.float64)
    gb = gb.astype(np.float64)
    for _ in range(steps):
        z = sig(gW[0] @ Q + gU[0] @ Q + gb[0])
        r = sig(gW[1] @ Q + gU[1] @ Q + gb[1])
        h = np.tanh(gW[2] @ Q + gU[2] @ (r * Q) + gb[2])
        Q = (1.0 - z) * Q + z * h
    return Q.astype(np.float32)


def _pack_windows(a, b, capA, capB, rng, wa=3, wb=5):
    """Assign rows (with group in-degrees a, b) of one shard to NW windows of
    64 slots, keeping window sums <= (capA[w], capB[w]).  Snake-deal by
    degree, then pairwise swap-repair of violations.  Returns positions."""
    n = len(a)
    order = np.argsort(-(a * wa + b * wb), kind="stable")
    wins = np.empty(n, np.int64)
    rnds = np.arange(n) // NW
    js = np.arange(n) % NW
    wins[order] = np.where(rnds % 2 == 0, js, NW - 1 - js)

    def sums():
        A = np.bincount(wins, weights=a, minlength=NW).astype(np.int64)
        B = np.bincount(wins, weights=b, minlength=NW).astype(np.int64)
        return A, B

    A, B = sums()
    members = [list(np.nonzero(wins == w)[0]) for w in range(NW)]
    al = a.tolist()
    bl = b.tolist()
    capAl, capBl = capA.tolist(), capB.tolist()
    stuck = np.zeros(NW, bool)
    resets = 0
    for _it in range(20000):
        vA = np.maximum(A - capA, 0)
        vB = np.maximum(B - capB, 0)
        v = vA + vB
        va = v.copy()
        va[stuck] = 0
        if va.max() == 0:
            if v.max() == 0 or stuck.all() or resets >= 6:
                break
            stuck[:] = False
            resets += 1
            continue
        w = int(np.argmax(va))
        overA = bool(vA[w] > 0)
        overB = bool(vB[w] > 0)
        rw = members[w]
        sc_w = sorted(rw, key=lambda r: -(al[r] * overA + bl[r] * overB))[:10]
        roomA = capA - A
        roomB = capB - B
        cand_w2 = np.argpartition(-(roomA + roomB), 10)[:10]
        cand_w2 = cand_w2[np.argsort(-(roomA + roomB)[cand_w2])]
        done = False
        for r in sc_w:
            ar, br = al[r], bl[r]
            for w2 in cand_w2:
                if w2 == w:
                    continue
                w2 = int(w2)
                r2i = sorted(
                    members[w2],
                    key=lambda x: al[x] * overA + bl[x] * overB,
                )[:10]
                vold = int(v[w] + v[w2])
                for r2 in r2i:
                    a2, b2 = al[r2], bl[r2]
                    nA_w, nB_w = A[w] - ar + a2, B[w] - br + b2
                    nA_2, nB_2 = A[w2] + ar - a2, B[w2] + br - b2
                    new = (max(nA_w - capAl[w], 0) + max(nB_w - capBl[w], 0)
                           + max(nA_2 - capAl[w2], 0) + max(nB_2 - capBl[w2], 0))
                    if new < vold:
                        wins[r], wins[r2] = w2, w
                        members[w].remove(r)
                        members[w2].remove(r2)
                        members[w].append(r2)
                        members[w2].append(r)
                        A[w], B[w] = nA_w, nB_w
                        A[w2], B[w2] = nA_2, nB_2
                        done = True
                        break
                if done:
                    break
            if done:
                break
        if not done:
            stuck[w] = True
    pos = np.empty(n, np.int64)
    for w in range(NW):
        rows = np.nonzero(wins == w)[0]
        pos[rows] = w * WROWS + np.arange(len(rows))
    return pos


def _prep_edges(row, col, val):
    """Host-side schedule. Returns (sched, per-core input arrays)."""
    # ---- window packing -> within-shard positions
    gcol = (col // NPC) >= ACORES
    a_deg = np.bincount(row[~gcol], minlength=N)
    b_deg = np.bincount(row[gcol], minlength=N)
    # shared overflow-window profile: last KA/KB windows get one extra chunk
    a_tot = a_deg.reshape(NC, NPC).sum(axis=1)
    b_tot = b_deg.reshape(NC, NPC).sum(axis=1)
    KA = max(0, -(-(int(a_tot.max()) + 256 - NW * TGT_A) // 128))
    KB = max(0, -(-(int(b_tot.max()) + 256 - NW * TGT_B) // 128))
    capA = np.full(NW, TGT_A, np.int64)
    capA[NW - KA :] = TGT_A + 128
    capB = np.full(NW, TGT_B, np.int64)
    capB[NW - KB :] = TGT_B + 128
    pos = np.empty(N, np.int64)
    rng = np.random.default_rng(0)
    for i in range(NC):
        lo, hi = i * NPC, (i + 1) * NPC
        best = None
        for wa, wb in ((3, 5), (1, 1), (5, 3), (1, 3)):
            p = _pack_windows(
                a_deg[lo:hi], b_deg[lo:hi], capA, capB, rng, wa, wb
            )
            w = p // WROWS
            A = np.bincount(w, weights=a_deg[lo:hi], minlength=NW)
            B = np.bincount(w, weights=b_deg[lo:hi], minlength=NW)
            score = (
                np.maximum(-(-A.astype(np.int64) // 128) - capA // 128, 0).sum()
                + np.maximum(-(-B.astype(np.int64) // 128) - capB // 128, 0).sum()
            )
            if best is None or score < best[0]:
                best = (score, p)
            if score == 0:
                break
        pos[lo:hi] = best[1]

    corei = row // NPC
    rl = pos[row]                       # scatter position within shard
    win = rl // WROWS
    rr = rl % WROWS
    tcol = (col // NPC) * RTP + pos[col]  # table row
    grp = (tcol >= SPLIT).astype(np.int64)

    # ---- merge exact duplicate (row, col) edges (S can only route a slot
    # to one destination row, so merging is valid only for identical rows)
    key = row * np.int64(N) + col
    order = np.argsort(key, kind="stable")
    key_s = key[order]
    uniq = np.empty(len(key_s), bool)
    uniq[0] = True
    uniq[1:] = key_s[1:] != key_s[:-1]
    seg_id = np.cumsum(uniq) - 1
    val_m = np.bincount(seg_id, weights=val[order].astype(np.float64))
    first = order[uniq]
    corei, win, rr, tcol, grp = (
        corei[first], win[first], rr[first], tcol[first], grp[first])
    val_m = val_m.astype(np.float32)

    # ---- shared chunk schedule: per (grp, win) max over cores
    counts = np.zeros((NC, 2, NW), np.int64)
    np.add.at(counts, (corei, grp, win), 1)
    CC = -(-counts // 128)
    CC = CC.max(axis=0)                 # [2, NW]
    CC[0] = np.maximum(CC[0], 1)        # every window needs >= 1 chunk
    baseA = np.zeros(NW + 1, np.int64)
    baseA[1:] = np.cumsum(CC[0])
    baseB = np.zeros(NW + 1, np.int64)
    baseB[1:] = np.cumsum(CC[1])
    NCHA, NCHB = int(baseA[-1]), int(baseB[-1])
    NCH = NCHA + NCHB
    NA, NB = NCHA * 128, NCHB * 128
    # unified S chunk ids, window-major (A then B within each window) so the
    # DVE S-build completes chunks in the order the spmm consumes them
    offW = np.zeros(NW + 1, np.int64)
    offW[1:] = np.cumsum(CC[0] + CC[1])

    idxa = np.zeros((NC, 128, NA // 16), np.int16)
    idxb = np.zeros((NC, 128, NB // 16), np.int16)
    valp = np.zeros((NC, 128, NCH), np.float16)
    rrp = np.full((NC, 128, NCH), 127.0, np.float16)

    for i in range(NC):
        for g, (base, idxg, idxoff) in enumerate(
            ((baseA, idxa, 0), (baseB, idxb, SPLIT))
        ):
            m = (corei == i) & (grp == g)
            ew, err = win[m], rr[m]
            etc = (tcol[m] - idxoff).astype(np.int16)
            ev = val_m[m]
            o = np.argsort(ew, kind="stable")
            ew, err, etc, ev = ew[o], err[o], etc[o], ev[o]
            winstart = np.searchsorted(ew, np.arange(NW))
            slot = base[ew] * 128 + (np.arange(ew.size) - winstart[ew])
            assert (slot < base[ew + 1] * 128).all()
            flat = np.zeros(base[-1] * 128, np.int16)
            flat[slot] = etc
            idxg[i][:16] = flat.reshape(-1, 16).T
            idxg[i] = np.tile(idxg[i][:16], (8, 1))
            p = slot % 128
            # unified chunk id: window-major
            gch = slot // 128                    # group-major chunk id
            loc = gch - base[ew]                 # chunk within window
            ch = offW[ew] + g * CC[0][ew] + loc
            valp[i, p, ch] = ev.astype(np.float16)
            rrp[i, p, ch] = err.astype(np.float16)

    sched = dict(
        CC=CC, baseA=baseA, baseB=baseB, NCHA=NCHA, NCHB=NCHB, offW=offW
    )
    return sched, pos, idxa, idxb, valp, rrp


def _build_program(sched):
    import concourse.bass as bass
    import concourse.tile as tile
    from concourse import bacc, mybir
    from contextlib import ExitStack

    F32, F16, BF16, I16 = (
        mybir.dt.float32, mybir.dt.float16, mybir.dt.bfloat16, mybir.dt.int16)
    baseA, baseB = sched["baseA"], sched["baseB"]
    NCHA, NCHB = sched["NCHA"], sched["NCHB"]
    offW = sched["offW"]
    CCA = sched["CC"][0]
    NCH = NCHA + NCHB
    NA, NB = NCHA * 128, NCHB * 128

    nc = bacc.Bacc(
        "TRN2", target_bir_lowering=False, debug=False,
        num_devices=(1 if SIM1 else NC),
    )
    xst_d = nc.dram_tensor("xst", [F, RTP], BF16, kind="ExternalInput")
    wio_d = nc.dram_tensor("wio", [F, 2 * F + WROWS], F16, kind="ExternalInput")
    idxab_d = nc.dram_tensor(
        "idxab", [128, (NA + NB) // 16], I16, kind="ExternalInput"
    )
    vr_d = nc.dram_tensor("vr", [128, 2 * NCH], F16, kind="ExternalInput")
    out_d = nc.dram_tensor("out", [F, RTP], F16, kind="ExternalOutput")

    # gather segments: SEGP row tiles each
    WQ = 128 // WROWS
    segs = []
    for p0 in range(0, NT, SEGP):
        p1 = min(p0 + SEGP, NT)
        segs.append((p0, p1, p0 * WQ, p1 * WQ))
    max_cha = max(int(baseA[w1] - baseA[w0]) for _, _, w0, w1 in segs)
    max_chb = max(int(baseB[w1] - baseB[w0]) for _, _, w0, w1 in segs)

    with tile.TileContext(nc) as tc, ExitStack() as ctx:
        const = ctx.enter_context(tc.tile_pool(name="const", bufs=1))
        big = ctx.enter_context(tc.tile_pool(name="big", bufs=1))
        tps = ctx.enter_context(tc.tile_pool(name="tps", bufs=2, space="PSUM"))
        tsh = ctx.enter_context(tc.tile_pool(name="tsh", bufs=14))
        accp = ctx.enter_context(tc.tile_pool(name="accp", bufs=4, space="PSUM"))
        msgp = ctx.enter_context(tc.tile_pool(name="msgp", bufs=3))
        h1p = ctx.enter_context(tc.tile_pool(name="h1p", bufs=3))
        dram = ctx.enter_context(tc.tile_pool(name="dram", bufs=1, space="DRAM"))

        # table-build / allgather chunks (tile ranges), segment-aligned,
        # small tail chunk so the layer transition drains fast
        CHB = [0, 4, 14, 24, 34, 44, NT]
        NCHK = len(CHB) - 1

        # --- inputs with no deps first (merged to few DMAs: each issue
        # holds the HWDGE unit ~625ns): fill DMA idle during table build
        wio_sb = const.tile([F, 2 * F + WROWS], F16)
        nc.sync.dma_start(wio_sb[:], wio_d[:, :])
        w1_sb = wio_sb[:, 0:F].bitcast(BF16)
        w2_sb = wio_sb[:, F : 2 * F].bitcast(BF16)
        iota_sb = wio_sb[:, 2 * F : 2 * F + WROWS]
        xst_c = []
        for g in range(NCHK):
            t0, t1 = CHB[g], CHB[g + 1]
            xt = big.tile([F, (t1 - t0) * 128], BF16, name=f"xst{g}")
            nc.sync.dma_start(xt[:], xst_d[:, t0 * 128 : t1 * 128])
            xst_c.append(xt)
        idxab_sb = big.tile([128, (NA + NB) // 16], I16)
        nc.sync.dma_start(idxab_sb[:], idxab_d[:, :])
        idxa_sb = idxab_sb[:, : NA // 16]
        idxb_sb = idxab_sb[:, NA // 16 :]
        vr_sb = big.tile([128, 2 * NCH], F16)
        nc.sync.dma_start(vr_sb[:], vr_d[:, :])
        valp_sb = vr_sb[:, :NCH]
        rrp_sb = vr_sb[:, NCH:]

        s_sb = big.tile([128, NCH * WROWS], F16)

        def build_s():
            # S[p, cid*64 + j] = (iota[j] == rr[p,cid]) * val[p,cid], on DVE
            SLAB = 128
            for c0 in range(0, NCH, SLAB):
                c1 = min(c0 + SLAB, NCH)
                nch = c1 - c0
                s_slab = s_sb[:, c0 * WROWS : c1 * WROWS]
                s3 = s_slab.rearrange("p (c j) -> p c j", j=WROWS)
                iota_b = iota_sb.unsqueeze(1).broadcast_to([128, nch, WROWS])
                rr_b = rrp_sb[:, c0:c1].unsqueeze(2).broadcast_to([128, nch, WROWS])
                val_b = valp_sb[:, c0:c1].unsqueeze(2).broadcast_to([128, nch, WROWS])
                nc.vector.tensor_tensor(
                    out=s3, in0=iota_b, in1=rr_b, op=mybir.AluOpType.is_equal
                )
                nc.vector.tensor_tensor(
                    out=s3, in0=s3, in1=val_b, op=mybir.AluOpType.mult
                )

        def build_tiles(src_sb, src_t0, w_sb, shard, t0, t1, dma_eng=None):
            """table rows [t0*128, t1*128) = (src^T)[rows] @ w, written as
            [64, 256]-per-tile fp16 (rows 2p, 2p+1 on partition p, so the
            DRAM writes are 512B-contiguous), four tiles per copy/DMA."""
            sh3 = shard.rearrange("(t q b) -> q t b", q=64, b=256)
            t = t0
            while t < t1:
                grp = min(4, t1 - t)
                ps = tps.tile([64, 1024], F32, tag="tp")
                for k in range(grp):
                    s0 = (t + k - src_t0) * 128
                    for par in range(2):
                        nc.tensor.matmul(
                            out=ps[:, k * 256 + par * 128 : k * 256 + (par + 1) * 128],
                            lhsT=src_sb[:, s0 + par : s0 + 128 : 2],
                            rhs=w_sb,
                            start=True, stop=True,
                        )
                sh = tsh.tile([64, 1024], F16, tag="sh")
                nc.scalar.activation(
                    sh[:, : grp * 256], ps[:, : grp * 256],
                    mybir.ActivationFunctionType.Copy,
                )
                sh_t = sh.rearrange("p (t b) -> p t b", b=256)
                (dma_eng or nc.sync).dma_start(
                    sh3[:, t : t + grp, :],
                    sh_t[:, :grp, :],
                )
                t += grp

        def all_gather(shard, table):
            if SIM1:
                for r in range(NC):
                    nc.sync.dma_start(
                        table[r * RTP * F : (r + 1) * RTP * F], shard[:]
                    )
            else:
                nc.gpsimd.collective_compute(
                    "AllGather",
                    mybir.AluOpType.bypass,
                    replica_groups=[list(range(NC))],
                    ins=[shard.opt()],
                    outs=[table.opt()],
                )

        # --- layer-1 table build (chunked for pipelining) + allgather
        _aspace = "Local" if SIM1 else "Shared"
        shard1 = dram.tile([RTP * F], F16, name="shard1")
        shard2 = dram.tile([RTP * F], F16, name="shard2")
        table1 = dram.tile([NC * RTP * F], F16, addr_space=_aspace, name="table1")
        table2 = dram.tile([NC * RTP * F], F16, addr_space=_aspace, name="table2")
        for g in range(NCHK):
            build_tiles(
                xst_c[g], CHB[g], w1_sb, shard1, CHB[g], CHB[g + 1],
            )
        build_s()
        all_gather(shard1, table1)

        def spmm(table, emit, interleave=None, per_tile=None):
            tbl = table.rearrange("(r f) -> r f", f=F)
            for si, (p0, p1, w0, w1) in enumerate(segs):
                ca0, ca1 = int(baseA[w0]), int(baseA[w1])
                cb0, cb1 = int(baseB[w0]), int(baseB[w1])
                na, nb = (ca1 - ca0) * 128, (cb1 - cb0) * 128
                msga = msgp.tile([128, max_cha, 128], F16, tag="msga")
                msgb = msgp.tile([128, max_chb, 128], F16, tag="msgb")
                if na:
                    nc.gpsimd.dma_gather(
                        out_ap=msga[:, : ca1 - ca0, :],
                        in_ap=tbl[:SPLIT, :],
                        idxs_ap=idxa_sb[:, ca0 * 8 : ca1 * 8],
                        num_idxs=na,
                        num_idxs_reg=na,
                        elem_size=F,
                        single_packet=False,
                    )
                if nb:
                    nc.gpsimd.dma_gather(
                        out_ap=msgb[:, : cb1 - cb0, :],
                        in_ap=tbl[SPLIT:, :],
                        idxs_ap=idxb_sb[:, cb0 * 8 : cb1 * 8],
                        num_idxs=nb,
                        num_idxs_reg=nb,
                        elem_size=F,
                        single_packet=False,
                    )
                emt = emit(si)
                for w in range(w0, w1):
                    acc = accp.tile([128, WROWS], F32, tag="acc")
                    nw_ch = int(
                        baseA[w + 1] - baseA[w] + baseB[w + 1] - baseB[w]
                    )
                    k = 0
                    for gc in range(int(baseA[w]), int(baseA[w + 1])):
                        cid = int(offW[w]) + (gc - int(baseA[w]))
                        nc.tensor.matmul(
                            out=acc[:],
                            lhsT=msga[:, gc - ca0, :],
                            rhs=s_sb[:, cid * WROWS : (cid + 1) * WROWS],
                            start=(k == 0),
                            stop=(k == nw_ch - 1),
                        )
                        k += 1
                    for gc in range(int(baseB[w]), int(baseB[w + 1])):
                        cid = int(offW[w]) + int(CCA[w]) + (gc - int(baseB[w]))
                        nc.tensor.matmul(
                            out=acc[:],
                            lhsT=msgb[:, gc - cb0, :],
                            rhs=s_sb[:, cid * WROWS : (cid + 1) * WROWS],
                            start=(k == 0),
                            stop=(k == nw_ch - 1),
                        )
                        k += 1
                    emt(w - w0, acc)
                    if per_tile is not None and w % WQ == WQ - 1:
                        per_tile(si, p0, w // WQ)
                if interleave is not None:
                    interleave(si, p0, p1)

        # --- layer 1: spmm -> h1T (bf16, per-segment tiles) -> table2 build
        h1tiles = {}

        def emit1(si):
            h1t = h1p.tile([F, SEGP * 128], BF16, tag="h1t")
            h1tiles[si] = h1t

            def e(wloc, acc):
                nc.scalar.activation(
                    h1t[:, wloc * WROWS : (wloc + 1) * WROWS],
                    acc[:],
                    mybir.ActivationFunctionType.Prelu,
                    alpha=SLOPE,
                )
            return e

        def interleave1(si, p0, p1):
            build_tiles(h1tiles[si], p0, w2_sb, shard2, p0, p1)

        spmm(table1, emit1, interleave1)
        all_gather(shard2, table2)

        # --- layer 2: spmm -> outT fp16 -> DRAM per segment
        out_sb = big.tile([F, RTP], F16)

        def emit2(si):
            p0, p1, w0, w1 = segs[si]

            def e(wloc, acc):
                w = w0 + wloc
                nc.scalar.activation(
                    out_sb[:, w * WROWS : (w + 1) * WROWS],
                    acc[:],
                    mybir.ActivationFunctionType.Prelu,
                    alpha=SLOPE,
                )
            return e

        def interleave2(si, p0, p1):
            if si < len(segs) - 1:
                nc.sync.dma_start(
                    out_d[:, p0 * 128 : p1 * 128],
                    out_sb[:, p0 * 128 : p1 * 128],
                )

        def emit2_last_tile(si, p0, t):
            # final segment: flush per tile so the last write overlaps the
            # remaining windows' compute
            if si == len(segs) - 1:
                nc.sync.dma_start(
                    out_d[:, t * 128 : (t + 1) * 128],
                    out_sb[:, t * 128 : (t + 1) * 128],
                )

        spmm(table2, emit2, interleave2, per_tile=emit2_last_tile)

    nc.compile()
    return nc


def kernel(
    features,
    adj_row,
    adj_col,
    adj_val,
    W1,
    g1_W,
    g1_U,
    g1_b,
    W2,
    g2_W,
    g2_U,
    g2_b,
    _run_kwargs=None,
):
    from concourse.bass_utils import run_bass_kernel_spmd

    X = np.asarray(features[T - 1], dtype=np.float32)
    row = np.asarray(adj_row[T - 1], dtype=np.int64)
    col = np.asarray(adj_col[T - 1], dtype=np.int64)
    val = np.asarray(adj_val[T - 1], dtype=np.float32)

    W1f = _evolve(np.asarray(W1), np.asarray(g1_W), np.asarray(g1_U), np.asarray(g1_b))
    W2f = _evolve(np.asarray(W2), np.asarray(g2_W), np.asarray(g2_U), np.asarray(g2_b))

    sched, pos, idxa, idxb, valp, rrp = _prep_edges(row, col, val)
    nc = _build_program(sched)

    # xsT per core: [128, RTP] bf16, column pos[v] = X[v]
    xst = np.zeros((NC, F, RTP), np_bf16)
    for i in range(NC):
        lo, hi = i * NPC, (i + 1) * NPC
        xst[i][:, pos[lo:hi]] = X[lo:hi].T.astype(np_bf16)

    # merged small inputs: [w1|w2] as bf16 bits in an f16 carrier + iota
    wio = np.zeros((F, 2 * F + WROWS), np.float16)
    wio[:, :F] = W1f.astype(np_bf16).view(np.float16)
    wio[:, F : 2 * F] = W2f.astype(np_bf16).view(np.float16)
    wio[:, 2 * F :] = np.arange(WROWS, dtype=np.float16)[None, :]
    idxab = np.concatenate([idxa, idxb], axis=2)
    vr = np.concatenate([valp, rrp], axis=2)

    in_maps = [
        {
            "xst": xst[i],
            "wio": wio,
            "idxab": idxab[i],
            "vr": vr[i],
        }
        for i in range(NC)
    ]
    res = run_bass_kernel_spmd(
        nc, in_maps, core_ids=list(range(NC)), **(_run_kwargs or {})
    )
    out = np.empty((N, F), np.float32)
    for i in range(NC):
        lo, hi = i * NPC, (i + 1) * NPC
        arr = res.results[i]["out"].astype(np.float32)  # [F, RTP]
        out[lo:hi] = arr[:, pos[lo:hi]].T
    if _run_kwargs:
        kernel.last_results = res
    return out


# revision 59
# speedup vs baseline: 1.0099x; 1.0074x over previous
"""EvolveGCN (2-layer) Trainium2 Bass kernel, 8-way sharded.

Algebraic reduction: the mat-GRU evolving the GCN weights is data-independent
and only h2[T-1] is returned, so the whole model collapses to

    W1* = matGRU^4(W1);  W2* = matGRU^4(W2)      (tiny host math)
    h1  = rrelu(A3 @ (X3 @ W1*));  out = rrelu(A3 @ (h1 @ W2*))

Device schedule (per core, nodes range-partitioned by original id):
  - X arrives transposed bf16 [128F, RTP]; table build is a plain matmul
    lhsT=xsT slice (even/odd row split so the fp16 DRAM shard writes are
    512B-contiguous), PSUM->fp16 via Activation copy.
  - AllGather replicates the fp16 table [50176, 128] to every core.
  - SWDGE dma_gather pulls per-edge messages (one 256B descriptor per edge)
    group A (table rows < 5*RTP) / group B split so indices fit int16.
  - Segment-sum runs on the tensor engine: per 64-row window, PSUM
    accumulates accT[128F, 64rows] += msg_chunk.T @ S_chunk, where S
    [128 edge-slots, 64 rows] carries val at (slot, row).  S is built
    on-device by the vector engine from packed val/rr arrays
    (S = (iota == rr) * val with 0-stride broadcast APs), not DMAed.
  - rrelu + down-cast is a single Prelu activation; layer-1 windows land in
    a transposed bf16 h1T tile that directly feeds the layer-2 table build
    (interleaved with layer-1's spmm); layer-2 windows land in a transposed
    fp16 out tile, written back per segment.
  - Host packs rows into windows (LPT on per-row A/B in-degree) so nearly
    every (window, group) hits its chunk budget exactly; the shared SPMD
    schedule is the per-window max over cores.
"""

import sys
import numpy as np

for _p in ("/opt/trn_rl_repo",):
    if _p not in sys.path:
        sys.path.insert(0, _p)

from ml_dtypes import bfloat16 as np_bf16

T, N, E, F = 4, 50000, 800000, 128
NC = 8
NPC = N // NC            # 6250 nodes per core
RTP = 6272               # padded rows per core (49 tiles of 128)
NT = RTP // 128          # 49 row tiles per core
WROWS = 64               # scatter window rows
NW = RTP // WROWS        # 98 windows per core
ACORES = 5               # table rows of cores [0,5) are group A
SPLIT = ACORES * RTP     # 31360 < 32768: both groups' indices fit int16
SLOPE = 11.0 / 48.0      # torch RReLU eval negative slope
SEGP = 2                 # row tiles per gather segment
TGT_A = 640              # per-window group-A edge target (5 chunks)
TGT_B = 384              # per-window group-B edge target (3 chunks)

SIM1 = False  # single-core, no-collective variant for TimelineSim
REPS = 1


def _evolve(W0, gW, gU, gb, steps=T):
    def sig(x):
        return 1.0 / (1.0 + np.exp(-x))

    Q = W0.astype(np.float64)
    gW = gW.astype(np.float64)
    gU = gU.astype(np.float64)
    gb = gb.astype(np.float64)
    for _ in range(steps):
        z = sig(gW[0] @ Q + gU[0] @ Q + gb[0])
        r = sig(gW[1] @ Q + gU[1] @ Q + gb[1])
        h = np.tanh(gW[2] @ Q + gU[2] @ (r * Q) + gb[2])
        Q = (1.0 - z) * Q + z * h
    return Q.astype(np.float32)


def _pack_windows(a, b, capA, capB, rng, wa=3, wb=5):
    """Assign rows (with group in-degrees a, b) of one shard to NW windows of
    64 slots, keeping window sums <= (capA[w], capB[w]).  Snake-deal by
    degree, then pairwise swap-repair of violations.  Returns positions."""
    n = len(a)
    order = np.argsort(-(a * wa + b * wb), kind="stable")
    wins = np.empty(n, np.int64)
    rnds = np.arange(n) // NW
    js = np.arange(n) % NW
    wins[order] = np.where(rnds % 2 == 0, js, NW - 1 - js)

    def sums():
        A = np.bincount(wins, weights=a, minlength=NW).astype(np.int64)
        B = np.bincount(wins, weights=b, minlength=NW).astype(np.int64)
        return A, B

    A, B = sums()
    members = [list(np.nonzero(wins == w)[0]) for w in range(NW)]
    al = a.tolist()
    bl = b.tolist()
    capAl, capBl = capA.tolist(), capB.tolist()
    stuck = np.zeros(NW, bool)
    resets = 0
    for _it in range(20000):
        vA = np.maximum(A - capA, 0)
        vB = np.maximum(B - capB, 0)
        v = vA + vB
        va = v.copy()
        va[stuck] = 0
        if va.max() == 0:
            if v.max() == 0 or stuck.all() or resets >= 6:
                break
            stuck[:] = False
            resets += 1
            continue
        w = int(np.argmax(va))
        overA = bool(vA[w] > 0)
        overB = bool(vB[w] > 0)
        K = 10 + 18 * resets  # widen the beam after each reset
        rw = members[w]
        sc_w = sorted(rw, key=lambda r: -(al[r] * overA + bl[r] * overB))[:K]
        roomA = capA - A
        roomB = capB - B
        cand_w2 = np.argpartition(-(roomA + roomB), min(K, NW - 1))[:K]
        cand_w2 = cand_w2[np.argsort(-(roomA + roomB)[cand_w2])]
        done = False
        for r in sc_w:
            ar, br = al[r], bl[r]
            for w2 in cand_w2:
                if w2 == w:
                    continue
                w2 = int(w2)
                r2i = sorted(
                    members[w2],
                    key=lambda x: al[x] * overA + bl[x] * overB,
                )[:K]
                vold = int(v[w] + v[w2])
                for r2 in r2i:
                    a2, b2 = al[r2], bl[r2]
                    nA_w, nB_w = A[w] - ar + a2, B[w] - br + b2
                    nA_2, nB_2 = A[w2] + ar - a2, B[w2] + br - b2
                    new = (max(nA_w - capAl[w], 0) + max(nB_w - capBl[w], 0)
                           + max(nA_2 - capAl[w2], 0) + max(nB_2 - capBl[w2], 0))
                    if new < vold:
                        wins[r], wins[r2] = w2, w
                        members[w].remove(r)
                        members[w2].remove(r2)
                        members[w].append(r2)
                        members[w2].append(r)
                        A[w], B[w] = nA_w, nB_w
                        A[w2], B[w2] = nA_2, nB_2
                        done = True
                        break
                if done:
                    break
            if done:
                break
        if not done:
            stuck[w] = True
    pos = np.empty(n, np.int64)
    for w in range(NW):
        rows = np.nonzero(wins == w)[0]
        pos[rows] = w * WROWS + np.arange(len(rows))
    return pos


def _prep_edges(row, col, val):
    """Host-side schedule. Returns (sched, per-core input arrays)."""
    # ---- window packing -> within-shard positions
    gcol = (col // NPC) >= ACORES
    a_deg = np.bincount(row[~gcol], minlength=N)
    b_deg = np.bincount(row[gcol], minlength=N)
    # shared overflow-window profile: last KA/KB windows get one extra chunk
    a_tot = a_deg.reshape(NC, NPC).sum(axis=1)
    b_tot = b_deg.reshape(NC, NPC).sum(axis=1)
    KA = max(0, -(-(int(a_tot.max()) + 256 - NW * TGT_A) // 128))
    KB = max(0, -(-(int(b_tot.max()) + 256 - NW * TGT_B) // 128))
    capA = np.full(NW, TGT_A, np.int64)
    capA[NW - KA :] = TGT_A + 128
    capB = np.full(NW, TGT_B, np.int64)
    capB[NW - KB :] = TGT_B + 128
    pos = np.empty(N, np.int64)
    rng = np.random.default_rng(0)
    for i in range(NC):
        lo, hi = i * NPC, (i + 1) * NPC
        best = None
        for wa, wb in ((3, 5), (1, 1), (5, 3), (1, 3), (2, 7), (7, 2), (1, 2), (4, 1)):
            p = _pack_windows(
                a_deg[lo:hi], b_deg[lo:hi], capA, capB, rng, wa, wb
            )
            w = p // WROWS
            A = np.bincount(w, weights=a_deg[lo:hi], minlength=NW)
            B = np.bincount(w, weights=b_deg[lo:hi], minlength=NW)
            score = (
                np.maximum(-(-A.astype(np.int64) // 128) - capA // 128, 0).sum()
                + np.maximum(-(-B.astype(np.int64) // 128) - capB // 128, 0).sum()
            )
            if best is None or score < best[0]:
                best = (score, p)
            if score == 0:
                break
        pos[lo:hi] = best[1]

    corei = row // NPC
    rl = pos[row]                       # scatter position within shard
    win = rl // WROWS
    rr = rl % WROWS
    tcol = (col // NPC) * RTP + pos[col]  # table row
    grp = (tcol >= SPLIT).astype(np.int64)

    # ---- merge exact duplicate (row, col) edges (S can only route a slot
    # to one destination row, so merging is valid only for identical rows)
    key = row * np.int64(N) + col
    order = np.argsort(key, kind="stable")
    key_s = key[order]
    uniq = np.empty(len(key_s), bool)
    uniq[0] = True
    uniq[1:] = key_s[1:] != key_s[:-1]
    seg_id = np.cumsum(uniq) - 1
    val_m = np.bincount(seg_id, weights=val[order].astype(np.float64))
    first = order[uniq]
    corei, win, rr, tcol, grp = (
        corei[first], win[first], rr[first], tcol[first], grp[first])
    val_m = val_m.astype(np.float32)

    # ---- shared chunk schedule: per (grp, win) max over cores
    counts = np.zeros((NC, 2, NW), np.int64)
    np.add.at(counts, (corei, grp, win), 1)
    CC = -(-counts // 128)
    CC = CC.max(axis=0)                 # [2, NW]
    CC[0] = np.maximum(CC[0], 1)        # every window needs >= 1 chunk
    baseA = np.zeros(NW + 1, np.int64)
    baseA[1:] = np.cumsum(CC[0])
    baseB = np.zeros(NW + 1, np.int64)
    baseB[1:] = np.cumsum(CC[1])
    NCHA, NCHB = int(baseA[-1]), int(baseB[-1])
    NCH = NCHA + NCHB
    NA, NB = NCHA * 128, NCHB * 128
    # unified S chunk ids, window-major (A then B within each window) so the
    # DVE S-build completes chunks in the order the spmm consumes them
    offW = np.zeros(NW + 1, np.int64)
    offW[1:] = np.cumsum(CC[0] + CC[1])

    idxa = np.zeros((NC, 128, NA // 16), np.int16)
    idxb = np.zeros((NC, 128, NB // 16), np.int16)
    valp = np.zeros((NC, 128, NCH), np.float16)
    rrp = np.full((NC, 128, NCH), 127.0, np.float16)

    for i in range(NC):
        for g, (base, idxg, idxoff) in enumerate(
            ((baseA, idxa, 0), (baseB, idxb, SPLIT))
        ):
            m = (corei == i) & (grp == g)
            ew, err = win[m], rr[m]
            etc = (tcol[m] - idxoff).astype(np.int16)
            ev = val_m[m]
            o = np.argsort(ew, kind="stable")
            ew, err, etc, ev = ew[o], err[o], etc[o], ev[o]
            winstart = np.searchsorted(ew, np.arange(NW))
            slot = base[ew] * 128 + (np.arange(ew.size) - winstart[ew])
            assert (slot < base[ew + 1] * 128).all()
            flat = np.zeros(base[-1] * 128, np.int16)
            flat[slot] = etc
            idxg[i][:16] = flat.reshape(-1, 16).T
            idxg[i] = np.tile(idxg[i][:16], (8, 1))
            p = slot % 128
            # unified chunk id: window-major
            gch = slot // 128                    # group-major chunk id
            loc = gch - base[ew]                 # chunk within window
            ch = offW[ew] + g * CC[0][ew] + loc
            valp[i, p, ch] = ev.astype(np.float16)
            rrp[i, p, ch] = err.astype(np.float16)

    sched = dict(
        CC=CC, baseA=baseA, baseB=baseB, NCHA=NCHA, NCHB=NCHB, offW=offW
    )
    return sched, pos, idxa, idxb, valp, rrp


def _build_program(sched):
    import concourse.bass as bass
    import concourse.tile as tile
    from concourse import bacc, mybir
    from contextlib import ExitStack

    F32, F16, BF16, I16 = (
        mybir.dt.float32, mybir.dt.float16, mybir.dt.bfloat16, mybir.dt.int16)
    baseA, baseB = sched["baseA"], sched["baseB"]
    NCHA, NCHB = sched["NCHA"], sched["NCHB"]
    offW = sched["offW"]
    CCA = sched["CC"][0]
    NCH = NCHA + NCHB
    NA, NB = NCHA * 128, NCHB * 128

    nc = bacc.Bacc(
        "TRN2", target_bir_lowering=False, debug=False,
        num_devices=(1 if SIM1 else NC),
    )
    xst_d = nc.dram_tensor("xst", [F, RTP], BF16, kind="ExternalInput")
    wio_d = nc.dram_tensor("wio", [F, 2 * F + WROWS], F16, kind="ExternalInput")
    idxab_d = nc.dram_tensor(
        "idxab", [128, (NA + NB) // 16], I16, kind="ExternalInput"
    )
    vr_d = nc.dram_tensor("vr", [128, 2 * NCH], F16, kind="ExternalInput")
    out_d = nc.dram_tensor("out", [F, RTP], F16, kind="ExternalOutput")

    # gather segments: SEGP row tiles each
    WQ = 128 // WROWS
    segs = []
    for p0 in range(0, NT, SEGP):
        p1 = min(p0 + SEGP, NT)
        segs.append((p0, p1, p0 * WQ, p1 * WQ))
    max_cha = max(int(baseA[w1] - baseA[w0]) for _, _, w0, w1 in segs)
    max_chb = max(int(baseB[w1] - baseB[w0]) for _, _, w0, w1 in segs)

    with tile.TileContext(nc) as tc, ExitStack() as ctx:
        const = ctx.enter_context(tc.tile_pool(name="const", bufs=1))
        big = ctx.enter_context(tc.tile_pool(name="big", bufs=1))
        tps = ctx.enter_context(tc.tile_pool(name="tps", bufs=2, space="PSUM"))
        tsh = ctx.enter_context(tc.tile_pool(name="tsh", bufs=14))
        accp = ctx.enter_context(tc.tile_pool(name="accp", bufs=4, space="PSUM"))
        msgp = ctx.enter_context(tc.tile_pool(name="msgp", bufs=3))
        h1p = ctx.enter_context(tc.tile_pool(name="h1p", bufs=3))
        dram = ctx.enter_context(tc.tile_pool(name="dram", bufs=1, space="DRAM"))

        # table-build / allgather chunks (tile ranges), segment-aligned,
        # small tail chunk so the layer transition drains fast
        CHB = [0, 4, 14, 24, 34, 44, NT]
        NCHK = len(CHB) - 1

        # --- inputs with no deps first (merged to few DMAs: each issue
        # holds the HWDGE unit ~625ns): fill DMA idle during table build
        wio_sb = const.tile([F, 2 * F + WROWS], F16)
        nc.sync.dma_start(wio_sb[:], wio_d[:, :])
        w1_sb = wio_sb[:, 0:F].bitcast(BF16)
        w2_sb = wio_sb[:, F : 2 * F].bitcast(BF16)
        iota_sb = wio_sb[:, 2 * F : 2 * F + WROWS]
        xst_c = []
        for g in range(NCHK):
            t0, t1 = CHB[g], CHB[g + 1]
            xt = big.tile([F, (t1 - t0) * 128], BF16, name=f"xst{g}")
            nc.sync.dma_start(xt[:], xst_d[:, t0 * 128 : t1 * 128])
            xst_c.append(xt)
        idxab_sb = big.tile([128, (NA + NB) // 16], I16)
        nc.sync.dma_start(idxab_sb[:], idxab_d[:, :])
        idxa_sb = idxab_sb[:, : NA // 16]
        idxb_sb = idxab_sb[:, NA // 16 :]
        vr_sb = big.tile([128, 2 * NCH], F16)
        nc.sync.dma_start(vr_sb[:], vr_d[:, :])
        valp_sb = vr_sb[:, :NCH]
        rrp_sb = vr_sb[:, NCH:]

        s_sb = big.tile([128, NCH * WROWS], F16)

        def build_s():
            # S[p, cid*64 + j] = (iota[j] == rr[p,cid]) * val[p,cid], on DVE
            SLAB = 128
            for c0 in range(0, NCH, SLAB):
                c1 = min(c0 + SLAB, NCH)
                nch = c1 - c0
                s_slab = s_sb[:, c0 * WROWS : c1 * WROWS]
                s3 = s_slab.rearrange("p (c j) -> p c j", j=WROWS)
                iota_b = iota_sb.unsqueeze(1).broadcast_to([128, nch, WROWS])
                rr_b = rrp_sb[:, c0:c1].unsqueeze(2).broadcast_to([128, nch, WROWS])
                val_b = valp_sb[:, c0:c1].unsqueeze(2).broadcast_to([128, nch, WROWS])
                nc.vector.tensor_tensor(
                    out=s3, in0=iota_b, in1=rr_b, op=mybir.AluOpType.is_equal
                )
                nc.vector.tensor_tensor(
                    out=s3, in0=s3, in1=val_b, op=mybir.AluOpType.mult
                )

        def build_tiles(src_sb, src_t0, w_sb, shard, t0, t1, dma_eng=None):
            """table rows [t0*128, t1*128) = (src^T)[rows] @ w, written as
            [64, 256]-per-tile fp16 (rows 2p, 2p+1 on partition p, so the
            DRAM writes are 512B-contiguous), four tiles per copy/DMA."""
            sh3 = shard.rearrange("(t q b) -> q t b", q=64, b=256)
            t = t0
            while t < t1:
                grp = min(4, t1 - t)
                ps = tps.tile([64, 1024], F32, tag="tp")
                for k in range(grp):
                    s0 = (t + k - src_t0) * 128
                    for par in range(2):
                        nc.tensor.matmul(
                            out=ps[:, k * 256 + par * 128 : k * 256 + (par + 1) * 128],
                            lhsT=src_sb[:, s0 + par : s0 + 128 : 2],
                            rhs=w_sb,
                            start=True, stop=True,
                        )
                sh = tsh.tile([64, 1024], F16, tag="sh")
                nc.scalar.activation(
                    sh[:, : grp * 256], ps[:, : grp * 256],
                    mybir.ActivationFunctionType.Copy,
                )
                sh_t = sh.rearrange("p (t b) -> p t b", b=256)
                (dma_eng or nc.sync).dma_start(
                    sh3[:, t : t + grp, :],
                    sh_t[:, :grp, :],
                )
                t += grp

        def all_gather(shard, table):
            if SIM1:
                for r in range(NC):
                    nc.sync.dma_start(
                        table[r * RTP * F : (r + 1) * RTP * F], shard[:]
                    )
            else:
                nc.gpsimd.collective_compute(
                    "AllGather",
                    mybir.AluOpType.bypass,
                    replica_groups=[list(range(NC))],
                    ins=[shard.opt()],
                    outs=[table.opt()],
                )

        # --- layer-1 table build (chunked for pipelining) + allgather
        _aspace = "Local" if SIM1 else "Shared"
        shard1 = dram.tile([RTP * F], F16, name="shard1")
        shard2 = dram.tile([RTP * F], F16, name="shard2")
        table1 = dram.tile([NC * RTP * F], F16, addr_space=_aspace, name="table1")
        table2 = dram.tile([NC * RTP * F], F16, addr_space=_aspace, name="table2")
        for g in range(NCHK):
            build_tiles(
                xst_c[g], CHB[g], w1_sb, shard1, CHB[g], CHB[g + 1],
            )
        build_s()
        all_gather(shard1, table1)

        def spmm(table, emit, interleave=None, per_tile=None):
            tbl = table.rearrange("(r f) -> r f", f=F)
            for si, (p0, p1, w0, w1) in enumerate(segs):
                ca0, ca1 = int(baseA[w0]), int(baseA[w1])
                cb0, cb1 = int(baseB[w0]), int(baseB[w1])
                na, nb = (ca1 - ca0) * 128, (cb1 - cb0) * 128
                msga = msgp.tile([128, max_cha, 128], F16, tag="msga")
                msgb = msgp.tile([128, max_chb, 128], F16, tag="msgb")
                if na:
                    nc.gpsimd.dma_gather(
                        out_ap=msga[:, : ca1 - ca0, :],
                        in_ap=tbl[:SPLIT, :],
                        idxs_ap=idxa_sb[:, ca0 * 8 : ca1 * 8],
                        num_idxs=na,
                        num_idxs_reg=na,
                        elem_size=F,
                        single_packet=False,
                    )
                if nb:
                    nc.gpsimd.dma_gather(
                        out_ap=msgb[:, : cb1 - cb0, :],
                        in_ap=tbl[SPLIT:, :],
                        idxs_ap=idxb_sb[:, cb0 * 8 : cb1 * 8],
                        num_idxs=nb,
                        num_idxs_reg=nb,
                        elem_size=F,
                        single_packet=False,
                    )
                emt = emit(si)
                for w in range(w0, w1):
                    acc = accp.tile([128, WROWS], F32, tag="acc")
                    nw_ch = int(
                        baseA[w + 1] - baseA[w] + baseB[w + 1] - baseB[w]
                    )
                    k = 0
                    for gc in range(int(baseA[w]), int(baseA[w + 1])):
                        cid = int(offW[w]) + (gc - int(baseA[w]))
                        nc.tensor.matmul(
                            out=acc[:],
                            lhsT=msga[:, gc - ca0, :],
                            rhs=s_sb[:, cid * WROWS : (cid + 1) * WROWS],
                            start=(k == 0),
                            stop=(k == nw_ch - 1),
                        )
                        k += 1
                    for gc in range(int(baseB[w]), int(baseB[w + 1])):
                        cid = int(offW[w]) + int(CCA[w]) + (gc - int(baseB[w]))
                        nc.tensor.matmul(
                            out=acc[:],
                            lhsT=msgb[:, gc - cb0, :],
                            rhs=s_sb[:, cid * WROWS : (cid + 1) * WROWS],
                            start=(k == 0),
                            stop=(k == nw_ch - 1),
                        )
                        k += 1
                    emt(w - w0, acc)
                    if per_tile is not None and w % WQ == WQ - 1:
                        per_tile(si, p0, w // WQ)
                if interleave is not None:
                    interleave(si, p0, p1)

        # --- layer 1: spmm -> h1T (bf16, per-segment tiles) -> table2 build
        h1tiles = {}

        def emit1(si):
            h1t = h1p.tile([F, SEGP * 128], BF16, tag="h1t")
            h1tiles[si] = h1t

            def e(wloc, acc):
                nc.scalar.activation(
                    h1t[:, wloc * WROWS : (wloc + 1) * WROWS],
                    acc[:],
                    mybir.ActivationFunctionType.Prelu,
                    alpha=SLOPE,
                )
            return e

        def interleave1(si, p0, p1):
            build_tiles(h1tiles[si], p0, w2_sb, shard2, p0, p1)

        spmm(table1, emit1, interleave1)
        all_gather(shard2, table2)

        # --- layer 2: spmm -> outT fp16 -> DRAM per segment
        out_sb = big.tile([F, RTP], F16)

        def emit2(si):
            p0, p1, w0, w1 = segs[si]

            def e(wloc, acc):
                w = w0 + wloc
                nc.scalar.activation(
                    out_sb[:, w * WROWS : (w + 1) * WROWS],
                    acc[:],
                    mybir.ActivationFunctionType.Prelu,
                    alpha=SLOPE,
                )
            return e

        def interleave2(si, p0, p1):
            if si < len(segs) - 1:
                nc.sync.dma_start(
                    out_d[:, p0 * 128 : p1 * 128],
                    out_sb[:, p0 * 128 : p1 * 128],
                )

        def emit2_last_tile(si, p0, t):
            # final segment: flush per tile so the last write overlaps the
            # remaining windows' compute
            if si == len(segs) - 1:
                nc.sync.dma_start(
                    out_d[:, t * 128 : (t + 1) * 128],
                    out_sb[:, t * 128 : (t + 1) * 128],
                )

        spmm(table2, emit2, interleave2, per_tile=emit2_last_tile)

    nc.compile()
    return nc


def kernel(
    features,
    adj_row,
    adj_col,
    adj_val,
    W1,
    g1_W,
    g1_U,
    g1_b,
    W2,
    g2_W,
    g2_U,
    g2_b,
    _run_kwargs=None,
):
    from concourse.bass_utils import run_bass_kernel_spmd

    X = np.asarray(features[T - 1], dtype=np.float32)
    row = np.asarray(adj_row[T - 1], dtype=np.int64)
    col = np.asarray(adj_col[T - 1], dtype=np.int64)
    val = np.asarray(adj_val[T - 1], dtype=np.float32)

    W1f = _evolve(np.asarray(W1), np.asarray(g1_W), np.asarray(g1_U), np.asarray(g1_b))
    W2f = _evolve(np.asarray(W2), np.asarray(g2_W), np.asarray(g2_U), np.asarray(g2_b))

    sched, pos, idxa, idxb, valp, rrp = _prep_edges(row, col, val)
    nc = _build_program(sched)

    # xsT per core: [128, RTP] bf16, column pos[v] = X[v]
    xst = np.zeros((NC, F, RTP), np_bf16)
    for i in range(NC):
        lo, hi = i * NPC, (i + 1) * NPC
        xst[i][:, pos[lo:hi]] = X[lo:hi].T.astype(np_bf16)

    # merged small inputs: [w1|w2] as bf16 bits in an f16 carrier + iota
    wio = np.zeros((F, 2 * F + WROWS), np.float16)
    wio[:, :F] = W1f.astype(np_bf16).view(np.float16)
    wio[:, F : 2 * F] = W2f.astype(np_bf16).view(np.float16)
    wio[:, 2 * F :] = np.arange(WROWS, dtype=np.float16)[None, :]
    idxab = np.concatenate([idxa, idxb], axis=2)
    vr = np.concatenate([valp, rrp], axis=2)

    in_maps = [
        {
            "xst": xst[i],
            "wio": wio,
            "idxab": idxab[i],
            "vr": vr[i],
        }
        for i in range(NC)
    ]
    res = run_bass_kernel_spmd(
        nc, in_maps, core_ids=list(range(NC)), **(_run_kwargs or {})
    )
    out = np.empty((N, F), np.float32)
    for i in range(NC):
        lo, hi = i * NPC, (i + 1) * NPC
        arr = res.results[i]["out"].astype(np.float32)  # [F, RTP]
        out[lo:hi] = arr[:, pos[lo:hi]].T
    if _run_kwargs:
        kernel.last_results = res
    return out


# revision 60
# speedup vs baseline: 1.0111x; 1.0012x over previous
"""EvolveGCN (2-layer) Trainium2 Bass kernel, 8-way sharded.

Algebraic reduction: the mat-GRU evolving the GCN weights is data-independent
and only h2[T-1] is returned, so the whole model collapses to

    W1* = matGRU^4(W1);  W2* = matGRU^4(W2)      (tiny host math)
    h1  = rrelu(A3 @ (X3 @ W1*));  out = rrelu(A3 @ (h1 @ W2*))

Device schedule (per core, nodes range-partitioned by original id):
  - X arrives transposed bf16 [128F, RTP]; table build is a plain matmul
    lhsT=xsT slice (even/odd row split so the fp16 DRAM shard writes are
    512B-contiguous), PSUM->fp16 via Activation copy.
  - AllGather replicates the fp16 table [50176, 128] to every core.
  - SWDGE dma_gather pulls per-edge messages (one 256B descriptor per edge)
    group A (table rows < 5*RTP) / group B split so indices fit int16.
  - Segment-sum runs on the tensor engine: per 64-row window, PSUM
    accumulates accT[128F, 64rows] += msg_chunk.T @ S_chunk, where S
    [128 edge-slots, 64 rows] carries val at (slot, row).  S is built
    on-device by the vector engine from packed val/rr arrays
    (S = (iota == rr) * val with 0-stride broadcast APs), not DMAed.
  - rrelu + down-cast is a single Prelu activation; layer-1 windows land in
    a transposed bf16 h1T tile that directly feeds the layer-2 table build
    (interleaved with layer-1's spmm); layer-2 windows land in a transposed
    fp16 out tile, written back per segment.
  - Host packs rows into windows (LPT on per-row A/B in-degree) so nearly
    every (window, group) hits its chunk budget exactly; the shared SPMD
    schedule is the per-window max over cores.
"""

import sys
import numpy as np

for _p in ("/opt/trn_rl_repo",):
    if _p not in sys.path:
        sys.path.insert(0, _p)

from ml_dtypes import bfloat16 as np_bf16

T, N, E, F = 4, 50000, 800000, 128
NC = 8
NPC = N // NC            # 6250 nodes per core
RTP = 6272               # padded rows per core (49 tiles of 128)
NT = RTP // 128          # 49 row tiles per core
WROWS = 64               # scatter window rows
NW = RTP // WROWS        # 98 windows per core
ACORES = 5               # table rows of cores [0,5) are group A
SPLIT = ACORES * RTP     # 31360 < 32768: both groups' indices fit int16
SLOPE = 11.0 / 48.0      # torch RReLU eval negative slope
SEGP = 3                 # row tiles per gather segment
TGT_A = 640              # per-window group-A edge target (5 chunks)
TGT_B = 384              # per-window group-B edge target (3 chunks)

SIM1 = False  # single-core, no-collective variant for TimelineSim
REPS = 1


def _evolve(W0, gW, gU, gb, steps=T):
    def sig(x):
        return 1.0 / (1.0 + np.exp(-x))

    Q = W0.astype(np.float64)
    gW = gW.astype(np.float64)
    gU = gU.astype(np.float64)
    gb = gb.astype(np.float64)
    for _ in range(steps):
        z = sig(gW[0] @ Q + gU[0] @ Q + gb[0])
        r = sig(gW[1] @ Q + gU[1] @ Q + gb[1])
        h = np.tanh(gW[2] @ Q + gU[2] @ (r * Q) + gb[2])
        Q = (1.0 - z) * Q + z * h
    return Q.astype(np.float32)


def _pack_windows(a, b, capA, capB, rng, wa=3, wb=5):
    """Assign rows (with group in-degrees a, b) of one shard to NW windows of
    64 slots, keeping window sums <= (capA[w], capB[w]).  Snake-deal by
    degree, then pairwise swap-repair of violations.  Returns positions."""
    n = len(a)
    order = np.argsort(-(a * wa + b * wb), kind="stable")
    wins = np.empty(n, np.int64)
    rnds = np.arange(n) // NW
    js = np.arange(n) % NW
    wins[order] = np.where(rnds % 2 == 0, js, NW - 1 - js)

    def sums():
        A = np.bincount(wins, weights=a, minlength=NW).astype(np.int64)
        B = np.bincount(wins, weights=b, minlength=NW).astype(np.int64)
        return A, B

    A, B = sums()
    members = [list(np.nonzero(wins == w)[0]) for w in range(NW)]
    al = a.tolist()
    bl = b.tolist()
    capAl, capBl = capA.tolist(), capB.tolist()
    stuck = np.zeros(NW, bool)
    resets = 0
    for _it in range(20000):
        vA = np.maximum(A - capA, 0)
        vB = np.maximum(B - capB, 0)
        v = vA + vB
        va = v.copy()
        va[stuck] = 0
        if va.max() == 0:
            if v.max() == 0 or stuck.all() or resets >= 6:
                break
            stuck[:] = False
            resets += 1
            continue
        w = int(np.argmax(va))
        overA = bool(vA[w] > 0)
        overB = bool(vB[w] > 0)
        K = 10 + 18 * resets  # widen the beam after each reset
        rw = members[w]
        sc_w = sorted(rw, key=lambda r: -(al[r] * overA + bl[r] * overB))[:K]
        roomA = capA - A
        roomB = capB - B
        cand_w2 = np.argpartition(-(roomA + roomB), min(K, NW - 1))[:K]
        cand_w2 = cand_w2[np.argsort(-(roomA + roomB)[cand_w2])]
        done = False
        for r in sc_w:
            ar, br = al[r], bl[r]
            for w2 in cand_w2:
                if w2 == w:
                    continue
                w2 = int(w2)
                r2i = sorted(
                    members[w2],
                    key=lambda x: al[x] * overA + bl[x] * overB,
                )[:K]
                vold = int(v[w] + v[w2])
                for r2 in r2i:
                    a2, b2 = al[r2], bl[r2]
                    nA_w, nB_w = A[w] - ar + a2, B[w] - br + b2
                    nA_2, nB_2 = A[w2] + ar - a2, B[w2] + br - b2
                    new = (max(nA_w - capAl[w], 0) + max(nB_w - capBl[w], 0)
                           + max(nA_2 - capAl[w2], 0) + max(nB_2 - capBl[w2], 0))
                    if new < vold:
                        wins[r], wins[r2] = w2, w
                        members[w].remove(r)
                        members[w2].remove(r2)
                        members[w].append(r2)
                        members[w2].append(r)
                        A[w], B[w] = nA_w, nB_w
                        A[w2], B[w2] = nA_2, nB_2
                        done = True
                        break
                if done:
                    break
            if done:
                break
        if not done:
            stuck[w] = True
    pos = np.empty(n, np.int64)
    for w in range(NW):
        rows = np.nonzero(wins == w)[0]
        pos[rows] = w * WROWS + np.arange(len(rows))
    return pos


def _prep_edges(row, col, val):
    """Host-side schedule. Returns (sched, per-core input arrays)."""
    # ---- window packing -> within-shard positions
    gcol = (col // NPC) >= ACORES
    a_deg = np.bincount(row[~gcol], minlength=N)
    b_deg = np.bincount(row[gcol], minlength=N)
    # shared overflow-window profile: last KA/KB windows get one extra chunk
    a_tot = a_deg.reshape(NC, NPC).sum(axis=1)
    b_tot = b_deg.reshape(NC, NPC).sum(axis=1)
    KA = max(0, -(-(int(a_tot.max()) + 256 - NW * TGT_A) // 128))
    KB = max(0, -(-(int(b_tot.max()) + 256 - NW * TGT_B) // 128))
    capA = np.full(NW, TGT_A, np.int64)
    capA[NW - KA :] = TGT_A + 128
    capB = np.full(NW, TGT_B, np.int64)
    capB[NW - KB :] = TGT_B + 128
    pos = np.empty(N, np.int64)
    rng = np.random.default_rng(0)
    for i in range(NC):
        lo, hi = i * NPC, (i + 1) * NPC
        best = None
        for wa, wb in ((3, 5), (1, 1), (5, 3), (1, 3), (2, 7), (7, 2), (1, 2), (4, 1)):
            p = _pack_windows(
                a_deg[lo:hi], b_deg[lo:hi], capA, capB, rng, wa, wb
            )
            w = p // WROWS
            A = np.bincount(w, weights=a_deg[lo:hi], minlength=NW)
            B = np.bincount(w, weights=b_deg[lo:hi], minlength=NW)
            score = (
                np.maximum(-(-A.astype(np.int64) // 128) - capA // 128, 0).sum()
                + np.maximum(-(-B.astype(np.int64) // 128) - capB // 128, 0).sum()
            )
            if best is None or score < best[0]:
                best = (score, p)
            if score == 0:
                break
        pos[lo:hi] = best[1]

    corei = row // NPC
    rl = pos[row]                       # scatter position within shard
    win = rl // WROWS
    rr = rl % WROWS
    tcol = (col // NPC) * RTP + pos[col]  # table row
    grp = (tcol >= SPLIT).astype(np.int64)

    # ---- merge exact duplicate (row, col) edges (S can only route a slot
    # to one destination row, so merging is valid only for identical rows)
    key = row * np.int64(N) + col
    order = np.argsort(key, kind="stable")
    key_s = key[order]
    uniq = np.empty(len(key_s), bool)
    uniq[0] = True
    uniq[1:] = key_s[1:] != key_s[:-1]
    seg_id = np.cumsum(uniq) - 1
    val_m = np.bincount(seg_id, weights=val[order].astype(np.float64))
    first = order[uniq]
    corei, win, rr, tcol, grp = (
        corei[first], win[first], rr[first], tcol[first], grp[first])
    val_m = val_m.astype(np.float32)

    # ---- shared chunk schedule: per (grp, win) max over cores
    counts = np.zeros((NC, 2, NW), np.int64)
    np.add.at(counts, (corei, grp, win), 1)
    CC = -(-counts // 128)
    CC = CC.max(axis=0)                 # [2, NW]
    CC[0] = np.maximum(CC[0], 1)        # every window needs >= 1 chunk
    baseA = np.zeros(NW + 1, np.int64)
    baseA[1:] = np.cumsum(CC[0])
    baseB = np.zeros(NW + 1, np.int64)
    baseB[1:] = np.cumsum(CC[1])
    NCHA, NCHB = int(baseA[-1]), int(baseB[-1])
    NCH = NCHA + NCHB
    NA, NB = NCHA * 128, NCHB * 128
    # unified S chunk ids, window-major (A then B within each window) so the
    # DVE S-build completes chunks in the order the spmm consumes them
    offW = np.zeros(NW + 1, np.int64)
    offW[1:] = np.cumsum(CC[0] + CC[1])

    idxa = np.zeros((NC, 128, NA // 16), np.int16)
    idxb = np.zeros((NC, 128, NB // 16), np.int16)
    valp = np.zeros((NC, 128, NCH), np.float16)
    rrp = np.full((NC, 128, NCH), 127.0, np.float16)

    for i in range(NC):
        for g, (base, idxg, idxoff) in enumerate(
            ((baseA, idxa, 0), (baseB, idxb, SPLIT))
        ):
            m = (corei == i) & (grp == g)
            ew, err = win[m], rr[m]
            etc = (tcol[m] - idxoff).astype(np.int16)
            ev = val_m[m]
            o = np.argsort(ew, kind="stable")
            ew, err, etc, ev = ew[o], err[o], etc[o], ev[o]
            winstart = np.searchsorted(ew, np.arange(NW))
            slot = base[ew] * 128 + (np.arange(ew.size) - winstart[ew])
            assert (slot < base[ew + 1] * 128).all()
            flat = np.zeros(base[-1] * 128, np.int16)
            flat[slot] = etc
            idxg[i][:16] = flat.reshape(-1, 16).T
            idxg[i] = np.tile(idxg[i][:16], (8, 1))
            p = slot % 128
            # unified chunk id: window-major
            gch = slot // 128                    # group-major chunk id
            loc = gch - base[ew]                 # chunk within window
            ch = offW[ew] + g * CC[0][ew] + loc
            valp[i, p, ch] = ev.astype(np.float16)
            rrp[i, p, ch] = err.astype(np.float16)

    sched = dict(
        CC=CC, baseA=baseA, baseB=baseB, NCHA=NCHA, NCHB=NCHB, offW=offW
    )
    return sched, pos, idxa, idxb, valp, rrp


def _build_program(sched):
    import concourse.bass as bass
    import concourse.tile as tile
    from concourse import bacc, mybir
    from contextlib import ExitStack

    F32, F16, BF16, I16 = (
        mybir.dt.float32, mybir.dt.float16, mybir.dt.bfloat16, mybir.dt.int16)
    baseA, baseB = sched["baseA"], sched["baseB"]
    NCHA, NCHB = sched["NCHA"], sched["NCHB"]
    offW = sched["offW"]
    CCA = sched["CC"][0]
    NCH = NCHA + NCHB
    NA, NB = NCHA * 128, NCHB * 128

    nc = bacc.Bacc(
        "TRN2", target_bir_lowering=False, debug=False,
        num_devices=(1 if SIM1 else NC),
    )
    xst_d = nc.dram_tensor("xst", [F, RTP], BF16, kind="ExternalInput")
    wio_d = nc.dram_tensor("wio", [F, 2 * F + WROWS], F16, kind="ExternalInput")
    idxab_d = nc.dram_tensor(
        "idxab", [128, (NA + NB) // 16], I16, kind="ExternalInput"
    )
    vr_d = nc.dram_tensor("vr", [128, 2 * NCH], F16, kind="ExternalInput")
    out_d = nc.dram_tensor("out", [F, RTP], F16, kind="ExternalOutput")

    # gather segments: SEGP row tiles each
    WQ = 128 // WROWS
    segs = []
    for p0 in range(0, NT, SEGP):
        p1 = min(p0 + SEGP, NT)
        segs.append((p0, p1, p0 * WQ, p1 * WQ))
    max_cha = max(int(baseA[w1] - baseA[w0]) for _, _, w0, w1 in segs)
    max_chb = max(int(baseB[w1] - baseB[w0]) for _, _, w0, w1 in segs)

    with tile.TileContext(nc) as tc, ExitStack() as ctx:
        const = ctx.enter_context(tc.tile_pool(name="const", bufs=1))
        big = ctx.enter_context(tc.tile_pool(name="big", bufs=1))
        tps = ctx.enter_context(tc.tile_pool(name="tps", bufs=2, space="PSUM"))
        tsh = ctx.enter_context(tc.tile_pool(name="tsh", bufs=14))
        accp = ctx.enter_context(tc.tile_pool(name="accp", bufs=4, space="PSUM"))
        msgp = ctx.enter_context(tc.tile_pool(name="msgp", bufs=3))
        h1p = ctx.enter_context(tc.tile_pool(name="h1p", bufs=3))
        dram = ctx.enter_context(tc.tile_pool(name="dram", bufs=1, space="DRAM"))

        # table-build / allgather chunks (tile ranges), segment-aligned,
        # small tail chunk so the layer transition drains fast
        CHB = [0, 4, 14, 24, 34, 44, NT]
        NCHK = len(CHB) - 1

        # --- inputs with no deps first (merged to few DMAs: each issue
        # holds the HWDGE unit ~625ns): fill DMA idle during table build
        wio_sb = const.tile([F, 2 * F + WROWS], F16)
        nc.sync.dma_start(wio_sb[:], wio_d[:, :])
        w1_sb = wio_sb[:, 0:F].bitcast(BF16)
        w2_sb = wio_sb[:, F : 2 * F].bitcast(BF16)
        iota_sb = wio_sb[:, 2 * F : 2 * F + WROWS]
        xst_c = []
        for g in range(NCHK):
            t0, t1 = CHB[g], CHB[g + 1]
            xt = big.tile([F, (t1 - t0) * 128], BF16, name=f"xst{g}")
            nc.sync.dma_start(xt[:], xst_d[:, t0 * 128 : t1 * 128])
            xst_c.append(xt)
        idxab_sb = big.tile([128, (NA + NB) // 16], I16)
        nc.sync.dma_start(idxab_sb[:], idxab_d[:, :])
        idxa_sb = idxab_sb[:, : NA // 16]
        idxb_sb = idxab_sb[:, NA // 16 :]
        vr_sb = big.tile([128, 2 * NCH], F16)
        nc.sync.dma_start(vr_sb[:], vr_d[:, :])
        valp_sb = vr_sb[:, :NCH]
        rrp_sb = vr_sb[:, NCH:]

        s_sb = big.tile([128, NCH * WROWS], F16)

        def build_s():
            # S[p, cid*64 + j] = (iota[j] == rr[p,cid]) * val[p,cid], on DVE
            SLAB = 128
            for c0 in range(0, NCH, SLAB):
                c1 = min(c0 + SLAB, NCH)
                nch = c1 - c0
                s_slab = s_sb[:, c0 * WROWS : c1 * WROWS]
                s3 = s_slab.rearrange("p (c j) -> p c j", j=WROWS)
                iota_b = iota_sb.unsqueeze(1).broadcast_to([128, nch, WROWS])
                rr_b = rrp_sb[:, c0:c1].unsqueeze(2).broadcast_to([128, nch, WROWS])
                val_b = valp_sb[:, c0:c1].unsqueeze(2).broadcast_to([128, nch, WROWS])
                nc.vector.tensor_tensor(
                    out=s3, in0=iota_b, in1=rr_b, op=mybir.AluOpType.is_equal
                )
                nc.vector.tensor_tensor(
                    out=s3, in0=s3, in1=val_b, op=mybir.AluOpType.mult
                )

        def build_tiles(src_sb, src_t0, w_sb, shard, t0, t1, dma_eng=None):
            """table rows [t0*128, t1*128) = (src^T)[rows] @ w, written as
            [64, 256]-per-tile fp16 (rows 2p, 2p+1 on partition p, so the
            DRAM writes are 512B-contiguous), four tiles per copy/DMA."""
            sh3 = shard.rearrange("(t q b) -> q t b", q=64, b=256)
            t = t0
            while t < t1:
                grp = min(4, t1 - t)
                ps = tps.tile([64, 1024], F32, tag="tp")
                for k in range(grp):
                    s0 = (t + k - src_t0) * 128
                    for par in range(2):
                        nc.tensor.matmul(
                            out=ps[:, k * 256 + par * 128 : k * 256 + (par + 1) * 128],
                            lhsT=src_sb[:, s0 + par : s0 + 128 : 2],
                            rhs=w_sb,
                            start=True, stop=True,
                        )
                sh = tsh.tile([64, 1024], F16, tag="sh")
                nc.scalar.activation(
                    sh[:, : grp * 256], ps[:, : grp * 256],
                    mybir.ActivationFunctionType.Copy,
                )
                sh_t = sh.rearrange("p (t b) -> p t b", b=256)
                (dma_eng or nc.sync).dma_start(
                    sh3[:, t : t + grp, :],
                    sh_t[:, :grp, :],
                )
                t += grp

        def all_gather(shard, table):
            if SIM1:
                for r in range(NC):
                    nc.sync.dma_start(
                        table[r * RTP * F : (r + 1) * RTP * F], shard[:]
                    )
            else:
                nc.gpsimd.collective_compute(
                    "AllGather",
                    mybir.AluOpType.bypass,
                    replica_groups=[list(range(NC))],
                    ins=[shard.opt()],
                    outs=[table.opt()],
                )

        # --- layer-1 table build (chunked for pipelining) + allgather
        _aspace = "Local" if SIM1 else "Shared"
        shard1 = dram.tile([RTP * F], F16, name="shard1")
        shard2 = dram.tile([RTP * F], F16, name="shard2")
        table1 = dram.tile([NC * RTP * F], F16, addr_space=_aspace, name="table1")
        table2 = dram.tile([NC * RTP * F], F16, addr_space=_aspace, name="table2")
        for g in range(NCHK):
            build_tiles(
                xst_c[g], CHB[g], w1_sb, shard1, CHB[g], CHB[g + 1],
            )
        build_s()
        all_gather(shard1, table1)

        def spmm(table, emit, interleave=None, per_tile=None):
            tbl = table.rearrange("(r f) -> r f", f=F)
            for si, (p0, p1, w0, w1) in enumerate(segs):
                ca0, ca1 = int(baseA[w0]), int(baseA[w1])
                cb0, cb1 = int(baseB[w0]), int(baseB[w1])
                na, nb = (ca1 - ca0) * 128, (cb1 - cb0) * 128
                msga = msgp.tile([128, max_cha, 128], F16, tag="msga")
                msgb = msgp.tile([128, max_chb, 128], F16, tag="msgb")
                if na:
                    nc.gpsimd.dma_gather(
                        out_ap=msga[:, : ca1 - ca0, :],
                        in_ap=tbl[:SPLIT, :],
                        idxs_ap=idxa_sb[:, ca0 * 8 : ca1 * 8],
                        num_idxs=na,
                        num_idxs_reg=na,
                        elem_size=F,
                        single_packet=False,
                    )
                if nb:
                    nc.gpsimd.dma_gather(
                        out_ap=msgb[:, : cb1 - cb0, :],
                        in_ap=tbl[SPLIT:, :],
                        idxs_ap=idxb_sb[:, cb0 * 8 : cb1 * 8],
                        num_idxs=nb,
                        num_idxs_reg=nb,
                        elem_size=F,
                        single_packet=False,
                    )
                emt = emit(si)
                for w in range(w0, w1):
                    acc = accp.tile([128, WROWS], F32, tag="acc")
                    nw_ch = int(
                        baseA[w + 1] - baseA[w] + baseB[w + 1] - baseB[w]
                    )
                    k = 0
                    for gc in range(int(baseA[w]), int(baseA[w + 1])):
                        cid = int(offW[w]) + (gc - int(baseA[w]))
                        nc.tensor.matmul(
                            out=acc[:],
                            lhsT=msga[:, gc - ca0, :],
                            rhs=s_sb[:, cid * WROWS : (cid + 1) * WROWS],
                            start=(k == 0),
                            stop=(k == nw_ch - 1),
                        )
                        k += 1
                    for gc in range(int(baseB[w]), int(baseB[w + 1])):
                        cid = int(offW[w]) + int(CCA[w]) + (gc - int(baseB[w]))
                        nc.tensor.matmul(
                            out=acc[:],
                            lhsT=msgb[:, gc - cb0, :],
                            rhs=s_sb[:, cid * WROWS : (cid + 1) * WROWS],
                            start=(k == 0),
                            stop=(k == nw_ch - 1),
                        )
                        k += 1
                    emt(w - w0, acc)
                    if per_tile is not None and w % WQ == WQ - 1:
                        per_tile(si, p0, w // WQ)
                if interleave is not None:
                    interleave(si, p0, p1)

        # --- layer 1: spmm -> h1T (bf16, per-segment tiles) -> table2 build
        h1tiles = {}

        def emit1(si):
            h1t = h1p.tile([F, SEGP * 128], BF16, tag="h1t")
            h1tiles[si] = h1t

            def e(wloc, acc):
                nc.scalar.activation(
                    h1t[:, wloc * WROWS : (wloc + 1) * WROWS],
                    acc[:],
                    mybir.ActivationFunctionType.Prelu,
                    alpha=SLOPE,
                )
            return e

        def interleave1(si, p0, p1):
            build_tiles(h1tiles[si], p0, w2_sb, shard2, p0, p1)

        spmm(table1, emit1, interleave1)
        all_gather(shard2, table2)

        # --- layer 2: spmm -> outT fp16 -> DRAM per segment
        out_sb = big.tile([F, RTP], F16)

        def emit2(si):
            p0, p1, w0, w1 = segs[si]

            def e(wloc, acc):
                w = w0 + wloc
                nc.scalar.activation(
                    out_sb[:, w * WROWS : (w + 1) * WROWS],
                    acc[:],
                    mybir.ActivationFunctionType.Prelu,
                    alpha=SLOPE,
                )
            return e

        def interleave2(si, p0, p1):
            if si < len(segs) - 1:
                nc.sync.dma_start(
                    out_d[:, p0 * 128 : p1 * 128],
                    out_sb[:, p0 * 128 : p1 * 128],
                )

        def emit2_last_tile(si, p0, t):
            # final segment: flush per tile so the last write overlaps the
            # remaining windows' compute
            if si == len(segs) - 1:
                nc.sync.dma_start(
                    out_d[:, t * 128 : (t + 1) * 128],
                    out_sb[:, t * 128 : (t + 1) * 128],
                )

        spmm(table2, emit2, interleave2, per_tile=emit2_last_tile)

    nc.compile()
    return nc


def kernel(
    features,
    adj_row,
    adj_col,
    adj_val,
    W1,
    g1_W,
    g1_U,
    g1_b,
    W2,
    g2_W,
    g2_U,
    g2_b,
    _run_kwargs=None,
):
    from concourse.bass_utils import run_bass_kernel_spmd

    X = np.asarray(features[T - 1], dtype=np.float32)
    row = np.asarray(adj_row[T - 1], dtype=np.int64)
    col = np.asarray(adj_col[T - 1], dtype=np.int64)
    val = np.asarray(adj_val[T - 1], dtype=np.float32)

    W1f = _evolve(np.asarray(W1), np.asarray(g1_W), np.asarray(g1_U), np.asarray(g1_b))
    W2f = _evolve(np.asarray(W2), np.asarray(g2_W), np.asarray(g2_U), np.asarray(g2_b))

    sched, pos, idxa, idxb, valp, rrp = _prep_edges(row, col, val)
    nc = _build_program(sched)

    # xsT per core: [128, RTP] bf16, column pos[v] = X[v]
    xst = np.zeros((NC, F, RTP), np_bf16)
    for i in range(NC):
        lo, hi = i * NPC, (i + 1) * NPC
        xst[i][:, pos[lo:hi]] = X[lo:hi].T.astype(np_bf16)

    # merged small inputs: [w1|w2] as bf16 bits in an f16 carrier + iota
    wio = np.zeros((F, 2 * F + WROWS), np.float16)
    wio[:, :F] = W1f.astype(np_bf16).view(np.float16)
    wio[:, F : 2 * F] = W2f.astype(np_bf16).view(np.float16)
    wio[:, 2 * F :] = np.arange(WROWS, dtype=np.float16)[None, :]
    idxab = np.concatenate([idxa, idxb], axis=2)
    vr = np.concatenate([valp, rrp], axis=2)

    in_maps = [
        {
            "xst": xst[i],
            "wio": wio,
            "idxab": idxab[i],
            "vr": vr[i],
        }
        for i in range(NC)
    ]
    res = run_bass_kernel_spmd(
        nc, in_maps, core_ids=list(range(NC)), **(_run_kwargs or {})
    )
    out = np.empty((N, F), np.float32)
    for i in range(NC):
        lo, hi = i * NPC, (i + 1) * NPC
        arr = res.results[i]["out"].astype(np.float32)  # [F, RTP]
        out[lo:hi] = arr[:, pos[lo:hi]].T
    if _run_kwargs:
        kernel.last_results = res
    return out


# revision 61
# speedup vs baseline: 1.0130x; 1.0018x over previous
"""EvolveGCN (2-layer) Trainium2 Bass kernel, 8-way sharded.

Algebraic reduction: the mat-GRU evolving the GCN weights is data-independent
and only h2[T-1] is returned, so the whole model collapses to

    W1* = matGRU^4(W1);  W2* = matGRU^4(W2)      (tiny host math)
    h1  = rrelu(A3 @ (X3 @ W1*));  out = rrelu(A3 @ (h1 @ W2*))

Device schedule (per core, nodes range-partitioned by original id):
  - X arrives transposed bf16 [128F, RTP]; table build is a plain matmul
    lhsT=xsT slice (even/odd row split so the fp16 DRAM shard writes are
    512B-contiguous), PSUM->fp16 via Activation copy.
  - AllGather replicates the fp16 table [50176, 128] to every core.
  - SWDGE dma_gather pulls per-edge messages (one 256B descriptor per edge)
    group A (table rows < 5*RTP) / group B split so indices fit int16.
  - Segment-sum runs on the tensor engine: per 64-row window, PSUM
    accumulates accT[128F, 64rows] += msg_chunk.T @ S_chunk, where S
    [128 edge-slots, 64 rows] carries val at (slot, row).  S is built
    on-device by the vector engine from packed val/rr arrays
    (S = (iota == rr) * val with 0-stride broadcast APs), not DMAed.
  - rrelu + down-cast is a single Prelu activation; layer-1 windows land in
    a transposed bf16 h1T tile that directly feeds the layer-2 table build
    (interleaved with layer-1's spmm); layer-2 windows land in a transposed
    fp16 out tile, written back per segment.
  - Host packs rows into windows (LPT on per-row A/B in-degree) so nearly
    every (window, group) hits its chunk budget exactly; the shared SPMD
    schedule is the per-window max over cores.
"""

import sys
import numpy as np

for _p in ("/opt/trn_rl_repo",):
    if _p not in sys.path:
        sys.path.insert(0, _p)

from ml_dtypes import bfloat16 as np_bf16

T, N, E, F = 4, 50000, 800000, 128
NC = 8
NPC = N // NC            # 6250 nodes per core
RTP = 6272               # padded rows per core (49 tiles of 128)
NT = RTP // 128          # 49 row tiles per core
WROWS = 64               # scatter window rows
NW = RTP // WROWS        # 98 windows per core
ACORES = 5               # table rows of cores [0,5) are group A
SPLIT = ACORES * RTP     # 31360 < 32768: both groups' indices fit int16
SLOPE = 11.0 / 48.0      # torch RReLU eval negative slope
SEGP = 3                 # row tiles per gather segment
TGT_A = 640              # per-window group-A edge target (5 chunks)
TGT_B = 384              # per-window group-B edge target (3 chunks)

SIM1 = False  # single-core, no-collective variant for TimelineSim
REPS = 1


def _evolve(W0, gW, gU, gb, steps=T):
    def sig(x):
        return 1.0 / (1.0 + np.exp(-x))

    Q = W0.astype(np.float64)
    gW = gW.astype(np.float64)
    gU = gU.astype(np.float64)
    gb = gb.astype(np.float64)
    for _ in range(steps):
        z = sig(gW[0] @ Q + gU[0] @ Q + gb[0])
        r = sig(gW[1] @ Q + gU[1] @ Q + gb[1])
        h = np.tanh(gW[2] @ Q + gU[2] @ (r * Q) + gb[2])
        Q = (1.0 - z) * Q + z * h
    return Q.astype(np.float32)


def _pack_windows(a, b, capA, capB, rng, wa=3, wb=5):
    """Assign rows (with group in-degrees a, b) of one shard to NW windows of
    64 slots, keeping window sums <= (capA[w], capB[w]).  Snake-deal by
    degree, then pairwise swap-repair of violations.  Returns positions."""
    n = len(a)
    order = np.argsort(-(a * wa + b * wb), kind="stable")
    wins = np.empty(n, np.int64)
    rnds = np.arange(n) // NW
    js = np.arange(n) % NW
    wins[order] = np.where(rnds % 2 == 0, js, NW - 1 - js)

    def sums():
        A = np.bincount(wins, weights=a, minlength=NW).astype(np.int64)
        B = np.bincount(wins, weights=b, minlength=NW).astype(np.int64)
        return A, B

    A, B = sums()
    members = [list(np.nonzero(wins == w)[0]) for w in range(NW)]
    al = a.tolist()
    bl = b.tolist()
    capAl, capBl = capA.tolist(), capB.tolist()
    stuck = np.zeros(NW, bool)
    resets = 0
    for _it in range(20000):
        vA = np.maximum(A - capA, 0)
        vB = np.maximum(B - capB, 0)
        v = vA + vB
        va = v.copy()
        va[stuck] = 0
        if va.max() == 0:
            if v.max() == 0 or stuck.all() or resets >= 6:
                break
            stuck[:] = False
            resets += 1
            continue
        w = int(np.argmax(va))
        overA = bool(vA[w] > 0)
        overB = bool(vB[w] > 0)
        K = 10 + 18 * resets  # widen the beam after each reset
        rw = members[w]
        sc_w = sorted(rw, key=lambda r: -(al[r] * overA + bl[r] * overB))[:K]
        roomA = capA - A
        roomB = capB - B
        cand_w2 = np.argpartition(-(roomA + roomB), min(K, NW - 1))[:K]
        cand_w2 = cand_w2[np.argsort(-(roomA + roomB)[cand_w2])]
        done = False
        for r in sc_w:
            ar, br = al[r], bl[r]
            for w2 in cand_w2:
                if w2 == w:
                    continue
                w2 = int(w2)
                r2i = sorted(
                    members[w2],
                    key=lambda x: al[x] * overA + bl[x] * overB,
                )[:K]
                vold = int(v[w] + v[w2])
                for r2 in r2i:
                    a2, b2 = al[r2], bl[r2]
                    nA_w, nB_w = A[w] - ar + a2, B[w] - br + b2
                    nA_2, nB_2 = A[w2] + ar - a2, B[w2] + br - b2
                    new = (max(nA_w - capAl[w], 0) + max(nB_w - capBl[w], 0)
                           + max(nA_2 - capAl[w2], 0) + max(nB_2 - capBl[w2], 0))
                    if new < vold:
                        wins[r], wins[r2] = w2, w
                        members[w].remove(r)
                        members[w2].remove(r2)
                        members[w].append(r2)
                        members[w2].append(r)
                        A[w], B[w] = nA_w, nB_w
                        A[w2], B[w2] = nA_2, nB_2
                        done = True
                        break
                if done:
                    break
            if done:
                break
        if not done:
            stuck[w] = True
    pos = np.empty(n, np.int64)
    for w in range(NW):
        rows = np.nonzero(wins == w)[0]
        pos[rows] = w * WROWS + np.arange(len(rows))
    return pos


def _prep_edges(row, col, val):
    """Host-side schedule. Returns (sched, per-core input arrays)."""
    # ---- window packing -> within-shard positions
    gcol = (col // NPC) >= ACORES
    a_deg = np.bincount(row[~gcol], minlength=N)
    b_deg = np.bincount(row[gcol], minlength=N)
    # shared overflow-window profile: last KA/KB windows get one extra chunk
    a_tot = a_deg.reshape(NC, NPC).sum(axis=1)
    b_tot = b_deg.reshape(NC, NPC).sum(axis=1)
    KA = max(0, -(-(int(a_tot.max()) + 64 - NW * TGT_A) // 128))
    KB = max(0, -(-(int(b_tot.max()) + 64 - NW * TGT_B) // 128))
    capA = np.full(NW, TGT_A, np.int64)
    capA[NW - KA :] = TGT_A + 128
    capB = np.full(NW, TGT_B, np.int64)
    capB[NW - KB :] = TGT_B + 128
    pos = np.empty(N, np.int64)
    rng = np.random.default_rng(0)
    for i in range(NC):
        lo, hi = i * NPC, (i + 1) * NPC
        best = None
        for wa, wb in ((3, 5), (1, 1), (5, 3), (1, 3), (2, 7), (7, 2), (1, 2), (4, 1)):
            p = _pack_windows(
                a_deg[lo:hi], b_deg[lo:hi], capA, capB, rng, wa, wb
            )
            w = p // WROWS
            A = np.bincount(w, weights=a_deg[lo:hi], minlength=NW)
            B = np.bincount(w, weights=b_deg[lo:hi], minlength=NW)
            score = (
                np.maximum(-(-A.astype(np.int64) // 128) - capA // 128, 0).sum()
                + np.maximum(-(-B.astype(np.int64) // 128) - capB // 128, 0).sum()
            )
            if best is None or score < best[0]:
                best = (score, p)
            if score == 0:
                break
        pos[lo:hi] = best[1]

    corei = row // NPC
    rl = pos[row]                       # scatter position within shard
    win = rl // WROWS
    rr = rl % WROWS
    tcol = (col // NPC) * RTP + pos[col]  # table row
    grp = (tcol >= SPLIT).astype(np.int64)

    # ---- merge exact duplicate (row, col) edges (S can only route a slot
    # to one destination row, so merging is valid only for identical rows)
    key = row * np.int64(N) + col
    order = np.argsort(key, kind="stable")
    key_s = key[order]
    uniq = np.empty(len(key_s), bool)
    uniq[0] = True
    uniq[1:] = key_s[1:] != key_s[:-1]
    seg_id = np.cumsum(uniq) - 1
    val_m = np.bincount(seg_id, weights=val[order].astype(np.float64))
    first = order[uniq]
    corei, win, rr, tcol, grp = (
        corei[first], win[first], rr[first], tcol[first], grp[first])
    val_m = val_m.astype(np.float32)

    # ---- shared chunk schedule: per (grp, win) max over cores
    counts = np.zeros((NC, 2, NW), np.int64)
    np.add.at(counts, (corei, grp, win), 1)
    CC = -(-counts // 128)
    CC = CC.max(axis=0)                 # [2, NW]
    CC[0] = np.maximum(CC[0], 1)        # every window needs >= 1 chunk
    baseA = np.zeros(NW + 1, np.int64)
    baseA[1:] = np.cumsum(CC[0])
    baseB = np.zeros(NW + 1, np.int64)
    baseB[1:] = np.cumsum(CC[1])
    NCHA, NCHB = int(baseA[-1]), int(baseB[-1])
    NCH = NCHA + NCHB
    NA, NB = NCHA * 128, NCHB * 128
    # unified S chunk ids, window-major (A then B within each window) so the
    # DVE S-build completes chunks in the order the spmm consumes them
    offW = np.zeros(NW + 1, np.int64)
    offW[1:] = np.cumsum(CC[0] + CC[1])

    idxa = np.zeros((NC, 128, NA // 16), np.int16)
    idxb = np.zeros((NC, 128, NB // 16), np.int16)
    valp = np.zeros((NC, 128, NCH), np.float16)
    rrp = np.full((NC, 128, NCH), 127.0, np.float16)

    for i in range(NC):
        for g, (base, idxg, idxoff) in enumerate(
            ((baseA, idxa, 0), (baseB, idxb, SPLIT))
        ):
            m = (corei == i) & (grp == g)
            ew, err = win[m], rr[m]
            etc = (tcol[m] - idxoff).astype(np.int16)
            ev = val_m[m]
            o = np.argsort(ew, kind="stable")
            ew, err, etc, ev = ew[o], err[o], etc[o], ev[o]
            winstart = np.searchsorted(ew, np.arange(NW))
            slot = base[ew] * 128 + (np.arange(ew.size) - winstart[ew])
            assert (slot < base[ew + 1] * 128).all()
            flat = np.zeros(base[-1] * 128, np.int16)
            flat[slot] = etc
            idxg[i][:16] = flat.reshape(-1, 16).T
            idxg[i] = np.tile(idxg[i][:16], (8, 1))
            p = slot % 128
            # unified chunk id: window-major
            gch = slot // 128                    # group-major chunk id
            loc = gch - base[ew]                 # chunk within window
            ch = offW[ew] + g * CC[0][ew] + loc
            valp[i, p, ch] = ev.astype(np.float16)
            rrp[i, p, ch] = err.astype(np.float16)

    sched = dict(
        CC=CC, baseA=baseA, baseB=baseB, NCHA=NCHA, NCHB=NCHB, offW=offW
    )
    return sched, pos, idxa, idxb, valp, rrp


def _build_program(sched):
    import concourse.bass as bass
    import concourse.tile as tile
    from concourse import bacc, mybir
    from contextlib import ExitStack

    F32, F16, BF16, I16 = (
        mybir.dt.float32, mybir.dt.float16, mybir.dt.bfloat16, mybir.dt.int16)
    baseA, baseB = sched["baseA"], sched["baseB"]
    NCHA, NCHB = sched["NCHA"], sched["NCHB"]
    offW = sched["offW"]
    CCA = sched["CC"][0]
    NCH = NCHA + NCHB
    NA, NB = NCHA * 128, NCHB * 128

    nc = bacc.Bacc(
        "TRN2", target_bir_lowering=False, debug=False,
        num_devices=(1 if SIM1 else NC),
    )
    xst_d = nc.dram_tensor("xst", [F, RTP], BF16, kind="ExternalInput")
    wio_d = nc.dram_tensor("wio", [F, 2 * F + WROWS], F16, kind="ExternalInput")
    idxab_d = nc.dram_tensor(
        "idxab", [128, (NA + NB) // 16], I16, kind="ExternalInput"
    )
    vr_d = nc.dram_tensor("vr", [128, 2 * NCH], F16, kind="ExternalInput")
    out_d = nc.dram_tensor("out", [F, RTP], F16, kind="ExternalOutput")

    # gather segments: SEGP row tiles each
    WQ = 128 // WROWS
    segs = []
    for p0 in range(0, NT, SEGP):
        p1 = min(p0 + SEGP, NT)
        segs.append((p0, p1, p0 * WQ, p1 * WQ))
    max_cha = max(int(baseA[w1] - baseA[w0]) for _, _, w0, w1 in segs)
    max_chb = max(int(baseB[w1] - baseB[w0]) for _, _, w0, w1 in segs)

    with tile.TileContext(nc) as tc, ExitStack() as ctx:
        const = ctx.enter_context(tc.tile_pool(name="const", bufs=1))
        big = ctx.enter_context(tc.tile_pool(name="big", bufs=1))
        tps = ctx.enter_context(tc.tile_pool(name="tps", bufs=2, space="PSUM"))
        tsh = ctx.enter_context(tc.tile_pool(name="tsh", bufs=14))
        accp = ctx.enter_context(tc.tile_pool(name="accp", bufs=4, space="PSUM"))
        msgp = ctx.enter_context(tc.tile_pool(name="msgp", bufs=3))
        h1p = ctx.enter_context(tc.tile_pool(name="h1p", bufs=3))
        dram = ctx.enter_context(tc.tile_pool(name="dram", bufs=1, space="DRAM"))

        # table-build / allgather chunks (tile ranges), segment-aligned,
        # small tail chunk so the layer transition drains fast
        CHB = [0, 4, 14, 24, 34, 44, NT]
        NCHK = len(CHB) - 1

        # --- inputs with no deps first (merged to few DMAs: each issue
        # holds the HWDGE unit ~625ns): fill DMA idle during table build
        wio_sb = const.tile([F, 2 * F + WROWS], F16)
        nc.sync.dma_start(wio_sb[:], wio_d[:, :])
        w1_sb = wio_sb[:, 0:F].bitcast(BF16)
        w2_sb = wio_sb[:, F : 2 * F].bitcast(BF16)
        iota_sb = wio_sb[:, 2 * F : 2 * F + WROWS]
        xst_c = []
        for g in range(NCHK):
            t0, t1 = CHB[g], CHB[g + 1]
            xt = big.tile([F, (t1 - t0) * 128], BF16, name=f"xst{g}")
            nc.sync.dma_start(xt[:], xst_d[:, t0 * 128 : t1 * 128])
            xst_c.append(xt)
        idxab_sb = big.tile([128, (NA + NB) // 16], I16)
        nc.sync.dma_start(idxab_sb[:], idxab_d[:, :])
        idxa_sb = idxab_sb[:, : NA // 16]
        idxb_sb = idxab_sb[:, NA // 16 :]
        vr_sb = big.tile([128, 2 * NCH], F16)
        nc.sync.dma_start(vr_sb[:], vr_d[:, :])
        valp_sb = vr_sb[:, :NCH]
        rrp_sb = vr_sb[:, NCH:]

        s_sb = big.tile([128, NCH * WROWS], F16)

        def build_s():
            # S[p, cid*64 + j] = (iota[j] == rr[p,cid]) * val[p,cid], on DVE
            SLAB = 128
            for c0 in range(0, NCH, SLAB):
                c1 = min(c0 + SLAB, NCH)
                nch = c1 - c0
                s_slab = s_sb[:, c0 * WROWS : c1 * WROWS]
                s3 = s_slab.rearrange("p (c j) -> p c j", j=WROWS)
                iota_b = iota_sb.unsqueeze(1).broadcast_to([128, nch, WROWS])
                rr_b = rrp_sb[:, c0:c1].unsqueeze(2).broadcast_to([128, nch, WROWS])
                val_b = valp_sb[:, c0:c1].unsqueeze(2).broadcast_to([128, nch, WROWS])
                nc.vector.tensor_tensor(
                    out=s3, in0=iota_b, in1=rr_b, op=mybir.AluOpType.is_equal
                )
                nc.vector.tensor_tensor(
                    out=s3, in0=s3, in1=val_b, op=mybir.AluOpType.mult
                )

        def build_tiles(src_sb, src_t0, w_sb, shard, t0, t1, dma_eng=None):
            """table rows [t0*128, t1*128) = (src^T)[rows] @ w, written as
            [64, 256]-per-tile fp16 (rows 2p, 2p+1 on partition p, so the
            DRAM writes are 512B-contiguous), four tiles per copy/DMA."""
            sh3 = shard.rearrange("(t q b) -> q t b", q=64, b=256)
            t = t0
            while t < t1:
                grp = min(4, t1 - t)
                ps = tps.tile([64, 1024], F32, tag="tp")
                for k in range(grp):
                    s0 = (t + k - src_t0) * 128
                    for par in range(2):
                        nc.tensor.matmul(
                            out=ps[:, k * 256 + par * 128 : k * 256 + (par + 1) * 128],
                            lhsT=src_sb[:, s0 + par : s0 + 128 : 2],
                            rhs=w_sb,
                            start=True, stop=True,
                        )
                sh = tsh.tile([64, 1024], F16, tag="sh")
                nc.scalar.activation(
                    sh[:, : grp * 256], ps[:, : grp * 256],
                    mybir.ActivationFunctionType.Copy,
                )
                sh_t = sh.rearrange("p (t b) -> p t b", b=256)
                (dma_eng or nc.sync).dma_start(
                    sh3[:, t : t + grp, :],
                    sh_t[:, :grp, :],
                )
                t += grp

        def all_gather(shard, table):
            if SIM1:
                for r in range(NC):
                    nc.sync.dma_start(
                        table[r * RTP * F : (r + 1) * RTP * F], shard[:]
                    )
            else:
                nc.gpsimd.collective_compute(
                    "AllGather",
                    mybir.AluOpType.bypass,
                    replica_groups=[list(range(NC))],
                    ins=[shard.opt()],
                    outs=[table.opt()],
                )

        # --- layer-1 table build (chunked for pipelining) + allgather
        _aspace = "Local" if SIM1 else "Shared"
        shard1 = dram.tile([RTP * F], F16, name="shard1")
        shard2 = dram.tile([RTP * F], F16, name="shard2")
        table1 = dram.tile([NC * RTP * F], F16, addr_space=_aspace, name="table1")
        table2 = dram.tile([NC * RTP * F], F16, addr_space=_aspace, name="table2")
        for g in range(NCHK):
            build_tiles(
                xst_c[g], CHB[g], w1_sb, shard1, CHB[g], CHB[g + 1],
            )
        build_s()
        all_gather(shard1, table1)

        def spmm(table, emit, interleave=None, per_tile=None):
            tbl = table.rearrange("(r f) -> r f", f=F)
            for si, (p0, p1, w0, w1) in enumerate(segs):
                ca0, ca1 = int(baseA[w0]), int(baseA[w1])
                cb0, cb1 = int(baseB[w0]), int(baseB[w1])
                na, nb = (ca1 - ca0) * 128, (cb1 - cb0) * 128
                msga = msgp.tile([128, max_cha, 128], F16, tag="msga")
                msgb = msgp.tile([128, max_chb, 128], F16, tag="msgb")
                if na:
                    nc.gpsimd.dma_gather(
                        out_ap=msga[:, : ca1 - ca0, :],
                        in_ap=tbl[:SPLIT, :],
                        idxs_ap=idxa_sb[:, ca0 * 8 : ca1 * 8],
                        num_idxs=na,
                        num_idxs_reg=na,
                        elem_size=F,
                        single_packet=False,
                    )
                if nb:
                    nc.gpsimd.dma_gather(
                        out_ap=msgb[:, : cb1 - cb0, :],
                        in_ap=tbl[SPLIT:, :],
                        idxs_ap=idxb_sb[:, cb0 * 8 : cb1 * 8],
                        num_idxs=nb,
                        num_idxs_reg=nb,
                        elem_size=F,
                        single_packet=False,
                    )
                emt = emit(si)
                for w in range(w0, w1):
                    acc = accp.tile([128, WROWS], F32, tag="acc")
                    nw_ch = int(
                        baseA[w + 1] - baseA[w] + baseB[w + 1] - baseB[w]
                    )
                    k = 0
                    for gc in range(int(baseA[w]), int(baseA[w + 1])):
                        cid = int(offW[w]) + (gc - int(baseA[w]))
                        nc.tensor.matmul(
                            out=acc[:],
                            lhsT=msga[:, gc - ca0, :],
                            rhs=s_sb[:, cid * WROWS : (cid + 1) * WROWS],
                            start=(k == 0),
                            stop=(k == nw_ch - 1),
                        )
                        k += 1
                    for gc in range(int(baseB[w]), int(baseB[w + 1])):
                        cid = int(offW[w]) + int(CCA[w]) + (gc - int(baseB[w]))
                        nc.tensor.matmul(
                            out=acc[:],
                            lhsT=msgb[:, gc - cb0, :],
                            rhs=s_sb[:, cid * WROWS : (cid + 1) * WROWS],
                            start=(k == 0),
                            stop=(k == nw_ch - 1),
                        )
                        k += 1
                    emt(w - w0, acc)
                    if per_tile is not None and w % WQ == WQ - 1:
                        per_tile(si, p0, w // WQ)
                if interleave is not None:
                    interleave(si, p0, p1)

        # --- layer 1: spmm -> h1T (bf16, per-segment tiles) -> table2 build
        h1tiles = {}

        def emit1(si):
            h1t = h1p.tile([F, SEGP * 128], BF16, tag="h1t")
            h1tiles[si] = h1t

            def e(wloc, acc):
                nc.scalar.activation(
                    h1t[:, wloc * WROWS : (wloc + 1) * WROWS],
                    acc[:],
                    mybir.ActivationFunctionType.Prelu,
                    alpha=SLOPE,
                )
            return e

        def interleave1(si, p0, p1):
            build_tiles(h1tiles[si], p0, w2_sb, shard2, p0, p1)

        spmm(table1, emit1, interleave1)
        all_gather(shard2, table2)

        # --- layer 2: spmm -> outT fp16 -> DRAM per segment
        out_sb = big.tile([F, RTP], F16)

        def emit2(si):
            p0, p1, w0, w1 = segs[si]

            def e(wloc, acc):
                w = w0 + wloc
                nc.scalar.activation(
                    out_sb[:, w * WROWS : (w + 1) * WROWS],
                    acc[:],
                    mybir.ActivationFunctionType.Prelu,
                    alpha=SLOPE,
                )
            return e

        def interleave2(si, p0, p1):
            if si < len(segs) - 1:
                nc.sync.dma_start(
                    out_d[:, p0 * 128 : p1 * 128],
                    out_sb[:, p0 * 128 : p1 * 128],
                )

        def emit2_last_tile(si, p0, t):
            # final segment: flush per tile so the last write overlaps the
            # remaining windows' compute
            if si == len(segs) - 1:
                nc.sync.dma_start(
                    out_d[:, t * 128 : (t + 1) * 128],
                    out_sb[:, t * 128 : (t + 1) * 128],
                )

        spmm(table2, emit2, interleave2, per_tile=emit2_last_tile)

    nc.compile()
    return nc


def kernel(
    features,
    adj_row,
    adj_col,
    adj_val,
    W1,
    g1_W,
    g1_U,
    g1_b,
    W2,
    g2_W,
    g2_U,
    g2_b,
    _run_kwargs=None,
):
    from concourse.bass_utils import run_bass_kernel_spmd

    X = np.asarray(features[T - 1], dtype=np.float32)
    row = np.asarray(adj_row[T - 1], dtype=np.int64)
    col = np.asarray(adj_col[T - 1], dtype=np.int64)
    val = np.asarray(adj_val[T - 1], dtype=np.float32)

    W1f = _evolve(np.asarray(W1), np.asarray(g1_W), np.asarray(g1_U), np.asarray(g1_b))
    W2f = _evolve(np.asarray(W2), np.asarray(g2_W), np.asarray(g2_U), np.asarray(g2_b))

    sched, pos, idxa, idxb, valp, rrp = _prep_edges(row, col, val)
    nc = _build_program(sched)

    # xsT per core: [128, RTP] bf16, column pos[v] = X[v]
    xst = np.zeros((NC, F, RTP), np_bf16)
    for i in range(NC):
        lo, hi = i * NPC, (i + 1) * NPC
        xst[i][:, pos[lo:hi]] = X[lo:hi].T.astype(np_bf16)

    # merged small inputs: [w1|w2] as bf16 bits in an f16 carrier + iota
    wio = np.zeros((F, 2 * F + WROWS), np.float16)
    wio[:, :F] = W1f.astype(np_bf16).view(np.float16)
    wio[:, F : 2 * F] = W2f.astype(np_bf16).view(np.float16)
    wio[:, 2 * F :] = np.arange(WROWS, dtype=np.float16)[None, :]
    idxab = np.concatenate([idxa, idxb], axis=2)
    vr = np.concatenate([valp, rrp], axis=2)

    in_maps = [
        {
            "xst": xst[i],
            "wio": wio,
            "idxab": idxab[i],
            "vr": vr[i],
        }
        for i in range(NC)
    ]
    res = run_bass_kernel_spmd(
        nc, in_maps, core_ids=list(range(NC)), **(_run_kwargs or {})
    )
    out = np.empty((N, F), np.float32)
    for i in range(NC):
        lo, hi = i * NPC, (i + 1) * NPC
        arr = res.results[i]["out"].astype(np.float32)  # [F, RTP]
        out[lo:hi] = arr[:, pos[lo:hi]].T
    if _run_kwargs:
        kernel.last_results = res
    return out


# revision 62
# speedup vs baseline: 1.0139x; 1.0009x over previous
"""EvolveGCN (2-layer) Trainium2 Bass kernel, 8-way sharded.

Algebraic reduction: the mat-GRU evolving the GCN weights is data-independent
and only h2[T-1] is returned, so the whole model collapses to

    W1* = matGRU^4(W1);  W2* = matGRU^4(W2)      (tiny host math)
    h1  = rrelu(A3 @ (X3 @ W1*));  out = rrelu(A3 @ (h1 @ W2*))

Device schedule (per core, nodes range-partitioned by original id):
  - X arrives transposed bf16 [128F, RTP]; table build is a plain matmul
    lhsT=xsT slice (even/odd row split so the fp16 DRAM shard writes are
    512B-contiguous), PSUM->fp16 via Activation copy.
  - AllGather replicates the fp16 table [50176, 128] to every core.
  - SWDGE dma_gather pulls per-edge messages (one 256B descriptor per edge)
    group A (table rows < 5*RTP) / group B split so indices fit int16.
  - Segment-sum runs on the tensor engine: per 64-row window, PSUM
    accumulates accT[128F, 64rows] += msg_chunk.T @ S_chunk, where S
    [128 edge-slots, 64 rows] carries val at (slot, row).  S is built
    on-device by the vector engine from packed val/rr arrays
    (S = (iota == rr) * val with 0-stride broadcast APs), not DMAed.
  - rrelu + down-cast is a single Prelu activation; layer-1 windows land in
    a transposed bf16 h1T tile that directly feeds the layer-2 table build
    (interleaved with layer-1's spmm); layer-2 windows land in a transposed
    fp16 out tile, written back per segment.
  - Host packs rows into windows (LPT on per-row A/B in-degree) so nearly
    every (window, group) hits its chunk budget exactly; the shared SPMD
    schedule is the per-window max over cores.
"""

import sys
import numpy as np

for _p in ("/opt/trn_rl_repo",):
    if _p not in sys.path:
        sys.path.insert(0, _p)

from ml_dtypes import bfloat16 as np_bf16

T, N, E, F = 4, 50000, 800000, 128
NC = 8
NPC = N // NC            # 6250 nodes per core
RTP = 6272               # padded rows per core (49 tiles of 128)
NT = RTP // 128          # 49 row tiles per core
WROWS = 64               # scatter window rows
NW = RTP // WROWS        # 98 windows per core
ACORES = 5               # table rows of cores [0,5) are group A
SPLIT = ACORES * RTP     # 31360 < 32768: both groups' indices fit int16
SLOPE = 11.0 / 48.0      # torch RReLU eval negative slope
SEGP = 3                 # row tiles per gather segment
TGT_A = 640              # per-window group-A edge target (5 chunks)
TGT_B = 384              # per-window group-B edge target (3 chunks)

SIM1 = False  # single-core, no-collective variant for TimelineSim
REPS = 1


def _evolve(W0, gW, gU, gb, steps=T):
    def sig(x):
        return 1.0 / (1.0 + np.exp(-x))

    Q = W0.astype(np.float64)
    gW = gW.astype(np.float64)
    gU = gU.astype(np.float64)
    gb = gb.astype(np.float64)
    for _ in range(steps):
        z = sig(gW[0] @ Q + gU[0] @ Q + gb[0])
        r = sig(gW[1] @ Q + gU[1] @ Q + gb[1])
        h = np.tanh(gW[2] @ Q + gU[2] @ (r * Q) + gb[2])
        Q = (1.0 - z) * Q + z * h
    return Q.astype(np.float32)


def _pack_windows(a, b, capA, capB, rng, wa=3, wb=5):
    """Assign rows (with group in-degrees a, b) of one shard to NW windows of
    64 slots, keeping window sums <= (capA[w], capB[w]).  Snake-deal by
    degree, then pairwise swap-repair of violations.  Returns positions."""
    n = len(a)
    order = np.argsort(-(a * wa + b * wb), kind="stable")
    wins = np.empty(n, np.int64)
    rnds = np.arange(n) // NW
    js = np.arange(n) % NW
    wins[order] = np.where(rnds % 2 == 0, js, NW - 1 - js)

    def sums():
        A = np.bincount(wins, weights=a, minlength=NW).astype(np.int64)
        B = np.bincount(wins, weights=b, minlength=NW).astype(np.int64)
        return A, B

    A, B = sums()
    members = [list(np.nonzero(wins == w)[0]) for w in range(NW)]
    al = a.tolist()
    bl = b.tolist()
    capAl, capBl = capA.tolist(), capB.tolist()
    stuck = np.zeros(NW, bool)
    resets = 0
    for _it in range(20000):
        vA = np.maximum(A - capA, 0)
        vB = np.maximum(B - capB, 0)
        v = vA + vB
        va = v.copy()
        va[stuck] = 0
        if va.max() == 0:
            if v.max() == 0 or stuck.all() or resets >= 6:
                break
            stuck[:] = False
            resets += 1
            continue
        w = int(np.argmax(va))
        overA = bool(vA[w] > 0)
        overB = bool(vB[w] > 0)
        K = 10 + 18 * resets  # widen the beam after each reset
        rw = members[w]
        sc_w = sorted(rw, key=lambda r: -(al[r] * overA + bl[r] * overB))[:K]
        roomA = capA - A
        roomB = capB - B
        cand_w2 = np.argpartition(-(roomA + roomB), min(K, NW - 1))[:K]
        cand_w2 = cand_w2[np.argsort(-(roomA + roomB)[cand_w2])]
        done = False
        for r in sc_w:
            ar, br = al[r], bl[r]
            for w2 in cand_w2:
                if w2 == w:
                    continue
                w2 = int(w2)
                r2i = sorted(
                    members[w2],
                    key=lambda x: al[x] * overA + bl[x] * overB,
                )[:K]
                vold = int(v[w] + v[w2])
                for r2 in r2i:
                    a2, b2 = al[r2], bl[r2]
                    nA_w, nB_w = A[w] - ar + a2, B[w] - br + b2
                    nA_2, nB_2 = A[w2] + ar - a2, B[w2] + br - b2
                    new = (max(nA_w - capAl[w], 0) + max(nB_w - capBl[w], 0)
                           + max(nA_2 - capAl[w2], 0) + max(nB_2 - capBl[w2], 0))
                    if new < vold:
                        wins[r], wins[r2] = w2, w
                        members[w].remove(r)
                        members[w2].remove(r2)
                        members[w].append(r2)
                        members[w2].append(r)
                        A[w], B[w] = nA_w, nB_w
                        A[w2], B[w2] = nA_2, nB_2
                        done = True
                        break
                if done:
                    break
            if done:
                break
        if not done:
            stuck[w] = True
    pos = np.empty(n, np.int64)
    for w in range(NW):
        rows = np.nonzero(wins == w)[0]
        pos[rows] = w * WROWS + np.arange(len(rows))
    return pos


def _prep_edges(row, col, val):
    """Host-side schedule. Returns (sched, per-core input arrays)."""
    # ---- window packing -> within-shard positions
    gcol = (col // NPC) >= ACORES
    a_deg = np.bincount(row[~gcol], minlength=N)
    b_deg = np.bincount(row[gcol], minlength=N)
    # shared overflow-window profile: last KA/KB windows get one extra chunk
    a_tot = a_deg.reshape(NC, NPC).sum(axis=1)
    b_tot = b_deg.reshape(NC, NPC).sum(axis=1)
    KA = max(0, -(-(int(a_tot.max()) - NW * TGT_A) // 128))
    KB = max(0, -(-(int(b_tot.max()) + 64 - NW * TGT_B) // 128))
    capA = np.full(NW, TGT_A, np.int64)
    capA[NW - KA :] = TGT_A + 128
    capB = np.full(NW, TGT_B, np.int64)
    capB[NW - KB :] = TGT_B + 128
    pos = np.empty(N, np.int64)
    rng = np.random.default_rng(0)
    for i in range(NC):
        lo, hi = i * NPC, (i + 1) * NPC
        best = None
        for wa, wb in ((3, 5), (1, 1), (5, 3), (1, 3), (2, 7), (7, 2), (1, 2), (4, 1)):
            p = _pack_windows(
                a_deg[lo:hi], b_deg[lo:hi], capA, capB, rng, wa, wb
            )
            w = p // WROWS
            A = np.bincount(w, weights=a_deg[lo:hi], minlength=NW)
            B = np.bincount(w, weights=b_deg[lo:hi], minlength=NW)
            score = (
                np.maximum(-(-A.astype(np.int64) // 128) - capA // 128, 0).sum()
                + np.maximum(-(-B.astype(np.int64) // 128) - capB // 128, 0).sum()
            )
            if best is None or score < best[0]:
                best = (score, p)
            if score == 0:
                break
        pos[lo:hi] = best[1]

    corei = row // NPC
    rl = pos[row]                       # scatter position within shard
    win = rl // WROWS
    rr = rl % WROWS
    tcol = (col // NPC) * RTP + pos[col]  # table row
    grp = (tcol >= SPLIT).astype(np.int64)

    # ---- merge exact duplicate (row, col) edges (S can only route a slot
    # to one destination row, so merging is valid only for identical rows)
    key = row * np.int64(N) + col
    order = np.argsort(key, kind="stable")
    key_s = key[order]
    uniq = np.empty(len(key_s), bool)
    uniq[0] = True
    uniq[1:] = key_s[1:] != key_s[:-1]
    seg_id = np.cumsum(uniq) - 1
    val_m = np.bincount(seg_id, weights=val[order].astype(np.float64))
    first = order[uniq]
    corei, win, rr, tcol, grp = (
        corei[first], win[first], rr[first], tcol[first], grp[first])
    val_m = val_m.astype(np.float32)

    # ---- shared chunk schedule: per (grp, win) max over cores
    counts = np.zeros((NC, 2, NW), np.int64)
    np.add.at(counts, (corei, grp, win), 1)
    CC = -(-counts // 128)
    CC = CC.max(axis=0)                 # [2, NW]
    CC[0] = np.maximum(CC[0], 1)        # every window needs >= 1 chunk
    baseA = np.zeros(NW + 1, np.int64)
    baseA[1:] = np.cumsum(CC[0])
    baseB = np.zeros(NW + 1, np.int64)
    baseB[1:] = np.cumsum(CC[1])
    NCHA, NCHB = int(baseA[-1]), int(baseB[-1])
    NCH = NCHA + NCHB
    NA, NB = NCHA * 128, NCHB * 128
    # unified S chunk ids, window-major (A then B within each window) so the
    # DVE S-build completes chunks in the order the spmm consumes them
    offW = np.zeros(NW + 1, np.int64)
    offW[1:] = np.cumsum(CC[0] + CC[1])

    idxa = np.zeros((NC, 128, NA // 16), np.int16)
    idxb = np.zeros((NC, 128, NB // 16), np.int16)
    valp = np.zeros((NC, 128, NCH), np.float16)
    rrp = np.full((NC, 128, NCH), 127.0, np.float16)

    for i in range(NC):
        for g, (base, idxg, idxoff) in enumerate(
            ((baseA, idxa, 0), (baseB, idxb, SPLIT))
        ):
            m = (corei == i) & (grp == g)
            ew, err = win[m], rr[m]
            etc = (tcol[m] - idxoff).astype(np.int16)
            ev = val_m[m]
            o = np.argsort(ew, kind="stable")
            ew, err, etc, ev = ew[o], err[o], etc[o], ev[o]
            winstart = np.searchsorted(ew, np.arange(NW))
            slot = base[ew] * 128 + (np.arange(ew.size) - winstart[ew])
            assert (slot < base[ew + 1] * 128).all()
            flat = np.zeros(base[-1] * 128, np.int16)
            flat[slot] = etc
            idxg[i][:16] = flat.reshape(-1, 16).T
            idxg[i] = np.tile(idxg[i][:16], (8, 1))
            p = slot % 128
            # unified chunk id: window-major
            gch = slot // 128                    # group-major chunk id
            loc = gch - base[ew]                 # chunk within window
            ch = offW[ew] + g * CC[0][ew] + loc
            valp[i, p, ch] = ev.astype(np.float16)
            rrp[i, p, ch] = err.astype(np.float16)

    sched = dict(
        CC=CC, baseA=baseA, baseB=baseB, NCHA=NCHA, NCHB=NCHB, offW=offW
    )
    return sched, pos, idxa, idxb, valp, rrp


def _build_program(sched):
    import concourse.bass as bass
    import concourse.tile as tile
    from concourse import bacc, mybir
    from contextlib import ExitStack

    F32, F16, BF16, I16 = (
        mybir.dt.float32, mybir.dt.float16, mybir.dt.bfloat16, mybir.dt.int16)
    baseA, baseB = sched["baseA"], sched["baseB"]
    NCHA, NCHB = sched["NCHA"], sched["NCHB"]
    offW = sched["offW"]
    CCA = sched["CC"][0]
    NCH = NCHA + NCHB
    NA, NB = NCHA * 128, NCHB * 128

    nc = bacc.Bacc(
        "TRN2", target_bir_lowering=False, debug=False,
        num_devices=(1 if SIM1 else NC),
    )
    xst_d = nc.dram_tensor("xst", [F, RTP], BF16, kind="ExternalInput")
    wio_d = nc.dram_tensor("wio", [F, 2 * F + WROWS], F16, kind="ExternalInput")
    idxab_d = nc.dram_tensor(
        "idxab", [128, (NA + NB) // 16], I16, kind="ExternalInput"
    )
    vr_d = nc.dram_tensor("vr", [128, 2 * NCH], F16, kind="ExternalInput")
    out_d = nc.dram_tensor("out", [F, RTP], F16, kind="ExternalOutput")

    # gather segments: SEGP row tiles each
    WQ = 128 // WROWS
    segs = []
    for p0 in range(0, NT, SEGP):
        p1 = min(p0 + SEGP, NT)
        segs.append((p0, p1, p0 * WQ, p1 * WQ))
    max_cha = max(int(baseA[w1] - baseA[w0]) for _, _, w0, w1 in segs)
    max_chb = max(int(baseB[w1] - baseB[w0]) for _, _, w0, w1 in segs)

    with tile.TileContext(nc) as tc, ExitStack() as ctx:
        const = ctx.enter_context(tc.tile_pool(name="const", bufs=1))
        big = ctx.enter_context(tc.tile_pool(name="big", bufs=1))
        tps = ctx.enter_context(tc.tile_pool(name="tps", bufs=2, space="PSUM"))
        tsh = ctx.enter_context(tc.tile_pool(name="tsh", bufs=14))
        accp = ctx.enter_context(tc.tile_pool(name="accp", bufs=4, space="PSUM"))
        msgp = ctx.enter_context(tc.tile_pool(name="msgp", bufs=3))
        h1p = ctx.enter_context(tc.tile_pool(name="h1p", bufs=3))
        dram = ctx.enter_context(tc.tile_pool(name="dram", bufs=1, space="DRAM"))

        # table-build / allgather chunks (tile ranges), segment-aligned,
        # small tail chunk so the layer transition drains fast
        CHB = [0, 4, 14, 24, 34, 44, NT]
        NCHK = len(CHB) - 1

        # --- inputs with no deps first (merged to few DMAs: each issue
        # holds the HWDGE unit ~625ns): fill DMA idle during table build
        wio_sb = const.tile([F, 2 * F + WROWS], F16)
        nc.sync.dma_start(wio_sb[:], wio_d[:, :])
        w1_sb = wio_sb[:, 0:F].bitcast(BF16)
        w2_sb = wio_sb[:, F : 2 * F].bitcast(BF16)
        iota_sb = wio_sb[:, 2 * F : 2 * F + WROWS]
        xst_c = []
        for g in range(NCHK):
            t0, t1 = CHB[g], CHB[g + 1]
            xt = big.tile([F, (t1 - t0) * 128], BF16, name=f"xst{g}")
            nc.sync.dma_start(xt[:], xst_d[:, t0 * 128 : t1 * 128])
            xst_c.append(xt)
        idxab_sb = big.tile([128, (NA + NB) // 16], I16)
        nc.sync.dma_start(idxab_sb[:], idxab_d[:, :])
        idxa_sb = idxab_sb[:, : NA // 16]
        idxb_sb = idxab_sb[:, NA // 16 :]
        vr_sb = big.tile([128, 2 * NCH], F16)
        nc.sync.dma_start(vr_sb[:], vr_d[:, :])
        valp_sb = vr_sb[:, :NCH]
        rrp_sb = vr_sb[:, NCH:]

        s_sb = big.tile([128, NCH * WROWS], F16)

        def build_s():
            # S[p, cid*64 + j] = (iota[j] == rr[p,cid]) * val[p,cid], on DVE
            SLAB = 128
            for c0 in range(0, NCH, SLAB):
                c1 = min(c0 + SLAB, NCH)
                nch = c1 - c0
                s_slab = s_sb[:, c0 * WROWS : c1 * WROWS]
                s3 = s_slab.rearrange("p (c j) -> p c j", j=WROWS)
                iota_b = iota_sb.unsqueeze(1).broadcast_to([128, nch, WROWS])
                rr_b = rrp_sb[:, c0:c1].unsqueeze(2).broadcast_to([128, nch, WROWS])
                val_b = valp_sb[:, c0:c1].unsqueeze(2).broadcast_to([128, nch, WROWS])
                nc.vector.tensor_tensor(
                    out=s3, in0=iota_b, in1=rr_b, op=mybir.AluOpType.is_equal
                )
                nc.vector.tensor_tensor(
                    out=s3, in0=s3, in1=val_b, op=mybir.AluOpType.mult
                )

        def build_tiles(src_sb, src_t0, w_sb, shard, t0, t1, dma_eng=None):
            """table rows [t0*128, t1*128) = (src^T)[rows] @ w, written as
            [64, 256]-per-tile fp16 (rows 2p, 2p+1 on partition p, so the
            DRAM writes are 512B-contiguous), four tiles per copy/DMA."""
            sh3 = shard.rearrange("(t q b) -> q t b", q=64, b=256)
            t = t0
            while t < t1:
                grp = min(4, t1 - t)
                ps = tps.tile([64, 1024], F32, tag="tp")
                for k in range(grp):
                    s0 = (t + k - src_t0) * 128
                    for par in range(2):
                        nc.tensor.matmul(
                            out=ps[:, k * 256 + par * 128 : k * 256 + (par + 1) * 128],
                            lhsT=src_sb[:, s0 + par : s0 + 128 : 2],
                            rhs=w_sb,
                            start=True, stop=True,
                        )
                sh = tsh.tile([64, 1024], F16, tag="sh")
                nc.scalar.activation(
                    sh[:, : grp * 256], ps[:, : grp * 256],
                    mybir.ActivationFunctionType.Copy,
                )
                sh_t = sh.rearrange("p (t b) -> p t b", b=256)
                (dma_eng or nc.sync).dma_start(
                    sh3[:, t : t + grp, :],
                    sh_t[:, :grp, :],
                )
                t += grp

        def all_gather(shard, table):
            if SIM1:
                for r in range(NC):
                    nc.sync.dma_start(
                        table[r * RTP * F : (r + 1) * RTP * F], shard[:]
                    )
            else:
                nc.gpsimd.collective_compute(
                    "AllGather",
                    mybir.AluOpType.bypass,
                    replica_groups=[list(range(NC))],
                    ins=[shard.opt()],
                    outs=[table.opt()],
                )

        # --- layer-1 table build (chunked for pipelining) + allgather
        _aspace = "Local" if SIM1 else "Shared"
        shard1 = dram.tile([RTP * F], F16, name="shard1")
        shard2 = dram.tile([RTP * F], F16, name="shard2")
        table1 = dram.tile([NC * RTP * F], F16, addr_space=_aspace, name="table1")
        table2 = dram.tile([NC * RTP * F], F16, addr_space=_aspace, name="table2")
        for g in range(NCHK):
            build_tiles(
                xst_c[g], CHB[g], w1_sb, shard1, CHB[g], CHB[g + 1],
            )
        build_s()
        all_gather(shard1, table1)

        def spmm(table, emit, interleave=None, per_tile=None):
            tbl = table.rearrange("(r f) -> r f", f=F)
            for si, (p0, p1, w0, w1) in enumerate(segs):
                ca0, ca1 = int(baseA[w0]), int(baseA[w1])
                cb0, cb1 = int(baseB[w0]), int(baseB[w1])
                na, nb = (ca1 - ca0) * 128, (cb1 - cb0) * 128
                msga = msgp.tile([128, max_cha, 128], F16, tag="msga")
                msgb = msgp.tile([128, max_chb, 128], F16, tag="msgb")
                if na:
                    nc.gpsimd.dma_gather(
                        out_ap=msga[:, : ca1 - ca0, :],
                        in_ap=tbl[:SPLIT, :],
                        idxs_ap=idxa_sb[:, ca0 * 8 : ca1 * 8],
                        num_idxs=na,
                        num_idxs_reg=na,
                        elem_size=F,
                        single_packet=False,
                    )
                if nb:
                    nc.gpsimd.dma_gather(
                        out_ap=msgb[:, : cb1 - cb0, :],
                        in_ap=tbl[SPLIT:, :],
                        idxs_ap=idxb_sb[:, cb0 * 8 : cb1 * 8],
                        num_idxs=nb,
                        num_idxs_reg=nb,
                        elem_size=F,
                        single_packet=False,
                    )
                emt = emit(si)
                for w in range(w0, w1):
                    acc = accp.tile([128, WROWS], F32, tag="acc")
                    nw_ch = int(
                        baseA[w + 1] - baseA[w] + baseB[w + 1] - baseB[w]
                    )
                    k = 0
                    for gc in range(int(baseA[w]), int(baseA[w + 1])):
                        cid = int(offW[w]) + (gc - int(baseA[w]))
                        nc.tensor.matmul(
                            out=acc[:],
                            lhsT=msga[:, gc - ca0, :],
                            rhs=s_sb[:, cid * WROWS : (cid + 1) * WROWS],
                            start=(k == 0),
                            stop=(k == nw_ch - 1),
                        )
                        k += 1
                    for gc in range(int(baseB[w]), int(baseB[w + 1])):
                        cid = int(offW[w]) + int(CCA[w]) + (gc - int(baseB[w]))
                        nc.tensor.matmul(
                            out=acc[:],
                            lhsT=msgb[:, gc - cb0, :],
                            rhs=s_sb[:, cid * WROWS : (cid + 1) * WROWS],
                            start=(k == 0),
                            stop=(k == nw_ch - 1),
                        )
                        k += 1
                    emt(w - w0, acc)
                    if per_tile is not None and w % WQ == WQ - 1:
                        per_tile(si, p0, w // WQ)
                if interleave is not None:
                    interleave(si, p0, p1)

        # --- layer 1: spmm -> h1T (bf16, per-segment tiles) -> table2 build
        h1tiles = {}

        def emit1(si):
            h1t = h1p.tile([F, SEGP * 128], BF16, tag="h1t")
            h1tiles[si] = h1t

            def e(wloc, acc):
                nc.scalar.activation(
                    h1t[:, wloc * WROWS : (wloc + 1) * WROWS],
                    acc[:],
                    mybir.ActivationFunctionType.Prelu,
                    alpha=SLOPE,
                )
            return e

        def interleave1(si, p0, p1):
            build_tiles(h1tiles[si], p0, w2_sb, shard2, p0, p1)

        spmm(table1, emit1, interleave1)
        all_gather(shard2, table2)

        # --- layer 2: spmm -> outT fp16 -> DRAM per segment
        out_sb = big.tile([F, RTP], F16)

        def emit2(si):
            p0, p1, w0, w1 = segs[si]

            def e(wloc, acc):
                w = w0 + wloc
                nc.scalar.activation(
                    out_sb[:, w * WROWS : (w + 1) * WROWS],
                    acc[:],
                    mybir.ActivationFunctionType.Prelu,
                    alpha=SLOPE,
                )
            return e

        def interleave2(si, p0, p1):
            if si < len(segs) - 1:
                nc.sync.dma_start(
                    out_d[:, p0 * 128 : p1 * 128],
                    out_sb[:, p0 * 128 : p1 * 128],
                )

        def emit2_last_tile(si, p0, t):
            # final segment: flush per tile so the last write overlaps the
            # remaining windows' compute
            if si == len(segs) - 1:
                nc.sync.dma_start(
                    out_d[:, t * 128 : (t + 1) * 128],
                    out_sb[:, t * 128 : (t + 1) * 128],
                )

        spmm(table2, emit2, interleave2, per_tile=emit2_last_tile)

    nc.compile()
    return nc


def kernel(
    features,
    adj_row,
    adj_col,
    adj_val,
    W1,
    g1_W,
    g1_U,
    g1_b,
    W2,
    g2_W,
    g2_U,
    g2_b,
    _run_kwargs=None,
):
    from concourse.bass_utils import run_bass_kernel_spmd

    X = np.asarray(features[T - 1], dtype=np.float32)
    row = np.asarray(adj_row[T - 1], dtype=np.int64)
    col = np.asarray(adj_col[T - 1], dtype=np.int64)
    val = np.asarray(adj_val[T - 1], dtype=np.float32)

    W1f = _evolve(np.asarray(W1), np.asarray(g1_W), np.asarray(g1_U), np.asarray(g1_b))
    W2f = _evolve(np.asarray(W2), np.asarray(g2_W), np.asarray(g2_U), np.asarray(g2_b))

    sched, pos, idxa, idxb, valp, rrp = _prep_edges(row, col, val)
    nc = _build_program(sched)

    # xsT per core: [128, RTP] bf16, column pos[v] = X[v]
    xst = np.zeros((NC, F, RTP), np_bf16)
    for i in range(NC):
        lo, hi = i * NPC, (i + 1) * NPC
        xst[i][:, pos[lo:hi]] = X[lo:hi].T.astype(np_bf16)

    # merged small inputs: [w1|w2] as bf16 bits in an f16 carrier + iota
    wio = np.zeros((F, 2 * F + WROWS), np.float16)
    wio[:, :F] = W1f.astype(np_bf16).view(np.float16)
    wio[:, F : 2 * F] = W2f.astype(np_bf16).view(np.float16)
    wio[:, 2 * F :] = np.arange(WROWS, dtype=np.float16)[None, :]
    idxab = np.concatenate([idxa, idxb], axis=2)
    vr = np.concatenate([valp, rrp], axis=2)

    in_maps = [
        {
            "xst": xst[i],
            "wio": wio,
            "idxab": idxab[i],
            "vr": vr[i],
        }
        for i in range(NC)
    ]
    res = run_bass_kernel_spmd(
        nc, in_maps, core_ids=list(range(NC)), **(_run_kwargs or {})
    )
    out = np.empty((N, F), np.float32)
    for i in range(NC):
        lo, hi = i * NPC, (i + 1) * NPC
        arr = res.results[i]["out"].astype(np.float32)  # [F, RTP]
        out[lo:hi] = arr[:, pos[lo:hi]].T
    if _run_kwargs:
        kernel.last_results = res
    return out
